# revision 10
# baseline (speedup 1.0000x reference)
"""Grouped-Query Attention (B=2, T=2048, E=2048, 16 Q heads / 4 KV heads, RoPE,
causal) as a Bass/Tile kernel on 8 Trainium2 NeuronCores.

Sharding: core c = 4*b + h handles batch b (of 2) and KV-head group h (of 4,
i.e. 4 q-heads + 1 kv head).  Host->device traffic is minimized (it dominates
the end-to-end time): inputs ship in fp16 with no cross-core duplication and
are reconstructed on device with AllGather collectives:

  - x ships token-quartered (core h gets quarter h of x[b]) -> 4-core AllGather
  - weights ship halved across the batch pair (cores h and h+4 need the same
    head-h weights) -> 2-core AllGather
  - rope tables + causal mask ship 1/8th per core -> 8-core AllGather
  - attention outputs y are AllGathered within each batch group so every core
    computes a distinct 512-row slice of the final out-projection; the output
    is 4x smaller and needs no host-side reduction.

On device everything is channel-major ([channel, token]); matmuls contract
along the partition axis with 512-wide moving operands, fp16 operands with
fp32 PSUM accumulation.
"""

import os
import sys

import numpy as np

try:
    import concourse.bass as bass
except ModuleNotFoundError:  # fresh grading dir: point at the in-container repo
    for p in ("/opt/trn_rl_repo", "/root/.axon_site/_ro/trn_rl_repo"):
        if os.path.isdir(p) and p not in sys.path:
            sys.path.insert(0, p)
    import concourse.bass as bass

from contextlib import ExitStack

import concourse.tile as tile
from concourse import bacc, mybir
from concourse.bass_utils import run_bass_kernel_spmd

# ---- problem constants (hardcoded per contract) ----
B, T, E = 2, 2048, 2048
N_QHEAD, N_KVHEAD = 16, 4
GROUP = N_QHEAD // N_KVHEAD          # 4 q heads per kv head
D = E // N_QHEAD                     # 128 head dim
KV = E // GROUP                      # 512 kv dim
ROPE_BASE = 10000.0
N_CORES = 8

P = 128                              # partitions
ET = E // P                          # 16 e-tiles
TT = T // P                          # 16 token tiles
TC = 512                             # moving-dim chunk (max for fp32 PSUM)
NTC = T // TC                        # 4 token chunks
TQ = T // GROUP                      # 512-token x quarter per core
WCOLS = (ET * GROUP + ET + ET + GROUP * ET) * P   # wq|wk|wv|wo = 20480
WHALF = WCOLS // 2                   # 10240 (= wq|wk and wv|wo exactly)
TME = (3 * T) // N_CORES             # 768 table/mask eighth columns

F32 = mybir.dt.float32
F16 = mybir.dt.float16

G_BATCH = [[0, 1, 2, 3], [4, 5, 6, 7]]           # 4-core batch groups
G_PAIR = [[0, 4], [1, 5], [2, 6], [3, 7]]        # same-head pairs
G_ALL = [list(range(N_CORES))]

_CACHE = {}


def _build_program():
    """Build + compile the (SPMD-identical) Bass program once per process."""
    if "nc" in _CACHE:
        return _CACHE["nc"]

    nc = bacc.Bacc("TRN2", target_bir_lowering=False, debug=False,
                   num_devices=N_CORES)

    dram = {}
    def din(name, shape, dt=F16):
        dram[name] = nc.dram_tensor(name, list(shape), dt,
                                    kind="ExternalInput").ap()
    din("xq", (ET, P, TQ))          # token-quarter h of x[b].T, (e, p, t)
    din("wh", (P, WHALF))           # batch-half of [wq|wk|wv|wo] tiles
    din("tm", (P, TME))             # eighth of [cosk|sink|mask4]
    din("ptm", (P, P))              # rope rotation matrix Pm^T (lhsT)
    din("ident", (P, P))            # identity (for PE transpose)
    din("bias6", (P, 6), F32)       # per-ctile biases: 4x bq, bk, bv
    din("bo4", (P, GROUP), F32)     # bo slice for this core's 4 j-tiles
    outt = nc.dram_tensor("outt", [GROUP, P, T], F16,
                          kind="ExternalOutput").ap()

    with tile.TileContext(nc) as tc:
        with ExitStack() as ctx, nc.allow_low_precision(
                reason="fp16 operands; accumulation stays fp32 in PSUM"):
            dpool = ctx.enter_context(tc.tile_pool(name="dram", bufs=1,
                                                   space="DRAM"))
            persist = ctx.enter_context(tc.tile_pool(name="persist", bufs=1))

            def ptile(shape, name, dt=F16):
                return persist.tile(shape, dt, tag=name, name=name)

            # ---------- DRAM bounce + gathered buffers ----------
            # CC can't read IO tensors -> bounce first.  The weight AllGather
            # goes first (cross-die pairs, D2D-bandwidth-bound); x is gathered
            # in two halves so phase 1 can start after the first half lands;
            # tables gather last (not needed until RoPE).
            TH = TQ // 2                              # 256-token half-quarters
            wb = dpool.tile([P, WHALF], F16)
            xb1 = dpool.tile([ET, P, TH], F16)
            xb2 = dpool.tile([ET, P, TH], F16)
            tb = dpool.tile([P, TME], F16)
            wg = dpool.tile([2, P, WHALF], F16)
            xg1 = dpool.tile([GROUP, ET, P, TH], F16)  # tokens m*512+[0,256)
            xg2 = dpool.tile([GROUP, ET, P, TH], F16)  # tokens m*512+[256,512)
            tg = dpool.tile([N_CORES, P, TME], F16)
            yb_g = [dpool.tile([P, T], F16, name=f"yb{g}")
                    for g in range(GROUP)]
            yg_g = [dpool.tile([GROUP, P, T], F16, name=f"yg{g}")
                    for g in range(GROUP)]

            nc.sync.dma_start(wb[:], dram["wh"][:])
            nc.sync.dma_start(xb1[:], dram["xq"][:, :, 0:TH])
            nc.sync.dma_start(xb2[:], dram["xq"][:, :, TH:TQ])
            nc.sync.dma_start(tb[:], dram["tm"][:])
            nc.gpsimd.collective_compute(
                "AllGather", mybir.AluOpType.bypass, replica_groups=G_PAIR,
                ins=[wb.opt()], outs=[wg.opt()])
            nc.gpsimd.collective_compute(
                "AllGather", mybir.AluOpType.bypass, replica_groups=G_BATCH,
                ins=[xb1.opt()], outs=[xg1.opt()])
            nc.gpsimd.collective_compute(
                "AllGather", mybir.AluOpType.bypass, replica_groups=G_BATCH,
                ins=[xb2.opt()], outs=[xg2.opt()])
            nc.gpsimd.collective_compute(
                "AllGather", mybir.AluOpType.bypass, replica_groups=G_ALL,
                ins=[tb.opt()], outs=[tg.opt()])

            # ---------- persistent SBUF tiles ----------
            wq_sb = ptile([P, ET * GROUP * P], "wq_sb")
            wk_sb = ptile([P, ET * P], "wk_sb")
            wv_sb = ptile([P, ET * P], "wv_sb")
            wo_sb = ptile([P, ET * GROUP * P], "wo_sb")
            bias6_sb = ptile([P, 8], "bias6_sb", F32)  # padded to 32B
            bo4_sb = ptile([P, GROUP], "bo4_sb", F32)
            ptm_sb = ptile([P, P], "ptm_sb")
            ident_sb = ptile([P, P], "ident_sb")
            qT_sb = ptile([P, GROUP * T], "qT_sb")    # 4 heads, channel-major
            kT_sb = ptile([P, T], "kT_sb")
            vT_sb = ptile([P, T], "vT_sb")
            vtok_sb = ptile([P, T], "vtok_sb")        # token-major v
            y_sb = ptile([P, GROUP * T], "y_sb")      # yT per head
            cos_sb = ptile([P, T], "cos_sb")
            sin_sb = ptile([P, T], "sin_sb")
            mask4_sb = ptile([P, GROUP * TC], "mask4_sb")
            ones1_sb = ptile([P, 8], "ones1_sb")
            onesr_sb = ptile([1, P], "onesr_sb")

            # pools (SBUF)
            xw = ctx.enter_context(tc.tile_pool(name="xw", bufs=2))    # x / y chunks
            ck = ctx.enter_context(tc.tile_pool(name="ck", bufs=2))    # exp tiles
            osb = ctx.enter_context(tc.tile_pool(name="osb", bufs=2))  # out staging
            # pools (PSUM): statically 4 + 4 = 8 banks
            pacc = ctx.enter_context(tc.tile_pool(name="pacc", bufs=4, space="PSUM"))
            pbig = ctx.enter_context(tc.tile_pool(name="pbig", bufs=2, space="PSUM"))

            # ---------- load constants ----------
            nc.sync.dma_start(bias6_sb[:, 0:6], dram["bias6"][:])
            nc.sync.dma_start(bo4_sb[:], dram["bo4"][:])
            nc.sync.dma_start(ptm_sb[:], dram["ptm"][:])
            nc.sync.dma_start(ident_sb[:], dram["ident"][:])
            nc.vector.memset(ones1_sb[:], 1.0)
            nc.vector.memset(onesr_sb[:], 1.0)
            # weights from the pair-gathered halves: [wq|wk] then [wv|wo]
            WQC = ET * GROUP * P                      # 8192
            for q4 in range(4):                       # split for DMA parallelism
                s = slice(q4 * WQC // 4, (q4 + 1) * WQC // 4)
                nc.sync.dma_start(wq_sb[:, s], wg[0][:, s])
            nc.sync.dma_start(wk_sb[:], wg[0][:, WQC:WQC + ET * P])
            nc.sync.dma_start(wv_sb[:], wg[1][:, 0:ET * P])
            for q4 in range(4):
                s = slice(q4 * WQC // 4, (q4 + 1) * WQC // 4)
                nc.sync.dma_start(wo_sb[:, s],
                                  wg[1][:, ET * P + q4 * WQC // 4:
                                        ET * P + (q4 + 1) * WQC // 4])
            # tables/mask from the 8-gathered eighths: per source core s the
            # slice holds [cos | sin | mask] of token range [s*256,(s+1)*256)
            TE8 = T // N_CORES                        # 256
            for s in range(N_CORES):
                cs = slice(s * TE8, (s + 1) * TE8)
                nc.sync.dma_start(cos_sb[:, cs], tg[s][:, 0:TE8])
                nc.sync.dma_start(sin_sb[:, cs], tg[s][:, TE8:2 * TE8])
                nc.sync.dma_start(mask4_sb[:, cs], tg[s][:, 2 * TE8:3 * TE8])

            # ---------- phase 1: QKV projections (channel-major) ----------
            # qT[c,t] = sum_e WqT[e,c] * xT[e,t]  (+bias at evacuation)
            XC = 256                      # x token-chunk width
            NXC = T // XC

            def proj_dst(ct):
                if ct < GROUP:
                    return qT_sb[:, ct * T:(ct + 1) * T]
                return (kT_sb if ct == GROUP else vT_sb)[:, :]

            for xc in [0, 2, 4, 6, 1, 3, 5, 7]:     # first-half chunks first
                mm = xc // 2
                x_sb = xw.tile([P, ET * XC], F16, tag="xw", name="x_sb")
                x3 = x_sb[:].rearrange("p (e t) -> p e t", e=ET)
                xgh = xg1 if xc % 2 == 0 else xg2
                xd = xgh[mm][:, :, :].rearrange("e p t -> p e t")
                for q4 in range(4):
                    nc.sync.dma_start(x3[:, q4 * 4:(q4 + 1) * 4, :],
                                      xd[:, q4 * 4:(q4 + 1) * 4, :])
                for half in range(2):          # <=3 live PSUM accums at a time
                    for ct3 in range(3):
                        ct = half * 3 + ct3
                        ppr = pacc.tile([P, XC], F32, tag="acc", name="ppr")
                        for e in range(ET):
                            if ct < GROUP:
                                lhs = wq_sb[:, (e * GROUP + ct) * P:
                                            (e * GROUP + ct + 1) * P]
                            elif ct == GROUP:
                                lhs = wk_sb[:, e * P:(e + 1) * P]
                            else:
                                lhs = wv_sb[:, e * P:(e + 1) * P]
                            nc.tensor.matmul(
                                ppr[:], lhs,
                                x_sb[:, e * XC:(e + 1) * XC],
                                start=(e == 0), stop=(e == ET - 1))
                        dst = proj_dst(ct)
                        nc.vector.tensor_scalar_add(
                            dst[:, xc * XC:(xc + 1) * XC], ppr[:],
                            bias6_sb[:, ct:ct + 1])

            # ---------- phase 1b: RoPE (shared tables; k scaled after) ------
            def rope(dst_full):
                for c in range(NTC):
                    cs = slice(c * TC, (c + 1) * TC)
                    rot_ps = pacc.tile([P, TC], F32, tag="acc", name="rot_ps")
                    nc.tensor.matmul(rot_ps[:], ptm_sb[:], dst_full[:, cs],
                                     start=True, stop=True)
                    tmp = osb.tile([P, TC], F32, tag="ost", name="tmp", bufs=4)
                    nc.vector.tensor_mul(tmp[:], rot_ps[:], sin_sb[:, cs])
                    nc.vector.tensor_mul(dst_full[:, cs],
                                         dst_full[:, cs], cos_sb[:, cs])
                    nc.vector.tensor_add(dst_full[:, cs],
                                         dst_full[:, cs], tmp[:])

            rope(kT_sb[:, :])
            # fold the 1/sqrt(D) score scale into k
            nc.vector.tensor_scalar_mul(kT_sb[:], kT_sb[:],
                                        float(1.0 / np.sqrt(D)))

            # ---------- phase 1c: v -> token-major via PE transpose ----------
            for j in range(TT):
                vps = pacc.tile([P, P], F16, tag="acc", name="vps")
                nc.tensor.transpose(vps[:], vT_sb[:, j * P:(j + 1) * P],
                                    ident_sb[:])
                nc.vector.tensor_copy(vtok_sb[:, j * P:(j + 1) * P], vps[:])

            # ---------- phase 2: causal attention per (head, tq-chunk) -------
            # transposed scores: sT[tk, tq] = kT_j^T . qT ; softmax over tk via
            # ones-matmul column sums; normalization folded in at the end.
            # rope of head h+1 (DVE-heavy) overlaps attention of head h
            # (PE-heavy) -- emitted just-in-time per head.
            for h in range(GROUP):
                rope(qT_sb[:, h * T:(h + 1) * T])
                for qc in range(NTC):
                    jmax = GROUP * qc + GROUP - 1
                    ng2 = 2 * (qc + 1)          # groups of 2 j-tiles
                    yps = pacc.tile([P, TC], F32, tag="acc", name="yps")
                    sps = pacc.tile([1, TC], F32, tag="acc", name="sps")

                    def scores(g):
                        # one [128,1024] PSUM tile holding 2 j-tiles' scores
                        spsum = pbig.tile([P, 2 * TC], F32, tag="big",
                                          name="spsum")
                        for sub in range(2):
                            j = 2 * g + sub
                            nc.tensor.matmul(
                                spsum[:, sub * TC:(sub + 1) * TC],
                                kT_sb[:, j * P:(j + 1) * P],
                                qT_sb[:, h * T + qc * TC:
                                      h * T + (qc + 1) * TC],
                                start=True, stop=True)
                        return spsum

                    # software-pipelined: scores of group g+1 are emitted
                    # before the exp/AV consumers of group g so the PE never
                    # sits behind the ACT exp in program order
                    spsum = scores(0)
                    for g in range(ng2):
                        nxt = scores(g + 1) if g + 1 < ng2 else None
                        eg = ck.tile([P, 2 * TC], F16, tag="ck", name="eg",
                                     bufs=3)
                        nc.scalar.activation(eg[:], spsum[:],
                                             mybir.ActivationFunctionType.Exp)
                        if g >= ng2 - 2:        # diagonal-straddling groups
                            half = g - (ng2 - 2)
                            nc.vector.tensor_mul(
                                eg[:], eg[:],
                                mask4_sb[:, half * 2 * TC:(half + 1) * 2 * TC])
                        for sub in range(2):
                            j = 2 * g + sub
                            nc.tensor.matmul(
                                yps[:], vtok_sb[:, j * P:(j + 1) * P],
                                eg[:, sub * TC:(sub + 1) * TC],
                                start=(j == 0), stop=(j == jmax))
                            nc.tensor.matmul(
                                sps[:], ones1_sb[:, 0:1],
                                eg[:, sub * TC:(sub + 1) * TC],
                                start=(j == 0), stop=(j == jmax))
                        spsum = nxt
                    # normalize: y /= colsum (broadcast 1/sum via K=1 matmul)
                    rec = osb.tile([1, TC], F16, tag="rec", name="rec", bufs=1)
                    nc.vector.reciprocal(rec[:], sps[:])
                    bps = pacc.tile([P, TC], F32, tag="acc", name="bps")
                    nc.tensor.matmul(bps[:], onesr_sb[:], rec[:],
                                     start=True, stop=True)
                    bcs = osb.tile([P, TC], F32, tag="bc", name="bcs", bufs=1)
                    nc.scalar.copy(bcs[:], bps[:])
                    nc.vector.tensor_mul(
                        y_sb[:, h * T + qc * TC: h * T + (qc + 1) * TC],
                        yps[:], bcs[:])
                # stage + gather this head's y now: the CC overlaps the next
                # head's attention; only head 3's gather sits on the tail
                nc.sync.dma_start(yb_g[h][:], y_sb[:, h * T:(h + 1) * T])
                nc.gpsimd.collective_compute(
                    "AllGather", mybir.AluOpType.bypass,
                    replica_groups=G_BATCH,
                    ins=[yb_g[h].opt()], outs=[yg_g[h].opt()])

            # ---------- phase 3: out-projection rows [h*512,(h+1)*512) ------
            # outT[j,t] = sum_c WoS[j,c] * yT[c,t] + bo[j]; y gathered from
            # all 4 cores of this batch group, streamed by token chunk.
            for c in range(NTC):
                ysb = xw.tile([P, ET * TC], F16, tag="ysb", name="ysb",
                              bufs=3)
                y4 = ysb[:].rearrange("p (h g t) -> p h g t", h=GROUP, g=GROUP)
                for g in range(GROUP):
                    src = yg_g[g][:, :, c * TC:(c + 1) * TC].rearrange(
                        "h d t -> d h t")
                    nc.sync.dma_start(y4[:, :, g, :], src)
                for jj in range(GROUP):
                    ops = pacc.tile([P, TC], F32, tag="acc", name="ops")
                    for ct in range(ET):
                        nc.tensor.matmul(
                            ops[:], wo_sb[:, (ct * GROUP + jj) * P:
                                          (ct * GROUP + jj + 1) * P],
                            ysb[:, ct * TC:(ct + 1) * TC],
                            start=(ct == 0), stop=(ct == ET - 1))
                    ost = osb.tile([P, TC], F16, tag="ost2", name="ost", bufs=4)
                    nc.vector.tensor_scalar_add(ost[:], ops[:],
                                                bo4_sb[:, jj:jj + 1])
                    nc.sync.dma_start(outt[jj][:, c * TC:(c + 1) * TC], ost[:])

    nc.compile()
    _CACHE["nc"] = nc
    return nc


def _host_inputs(x, Wq, bq, Wk, bk, Wv, bv, Wo, bo):
    """Per-core input dicts (fp16 payloads, layouts matching the DRAM decls)."""
    f = np.float32
    h16 = np.float16
    i = np.arange(1, D // 2 + 1, dtype=np.float64)
    thetas = ROPE_BASE ** (-2.0 * (i - 1.0) / D)
    ang = np.arange(1, T + 1, dtype=np.float64)[:, None] * thetas      # [T, D/2]
    cos = np.concatenate([np.cos(ang), np.cos(ang)], axis=1).T.astype(h16)
    sin = np.concatenate([np.sin(ang), np.sin(ang)], axis=1).T.astype(h16)

    Pm = np.zeros((D, D), h16)
    for d in range(D // 2):
        Pm[d, d + D // 2] = -1.0
        Pm[d + D // 2, d] = 1.0
    ptm = np.ascontiguousarray(Pm.T)
    ident = np.eye(P, dtype=h16)

    pcol = np.arange(P)[:, None]
    fcol = np.arange(TC)[None, :]
    mask4 = np.concatenate(
        [(pcol <= fcol - P * r).astype(h16) for r in range(GROUP)], axis=1)

    tmcat = np.concatenate([cos, sin, mask4], axis=1)       # [P, 3T]
    TE8 = T // N_CORES

    # per-head weight blocks [wq|wk|wv|wo] -> [P, 20480] fp16
    wcat_h = []
    for h in range(GROUP):
        WqS = Wq[h * KV:(h + 1) * KV, :]                                # [512, E]
        wq = WqS.T.reshape(ET, P, GROUP, P).transpose(1, 0, 2, 3).reshape(P, -1)
        WkS = Wk[h * D:(h + 1) * D, :]
        wk = WkS.T.reshape(ET, P, P).transpose(1, 0, 2).reshape(P, -1)
        WvS = Wv[h * D:(h + 1) * D, :]
        wv = WvS.T.reshape(ET, P, P).transpose(1, 0, 2).reshape(P, -1)
        WoS = Wo[h * KV:(h + 1) * KV, :]                                # [512, E]
        wo = WoS.reshape(GROUP, P, ET, P).transpose(3, 2, 0, 1).reshape(P, -1)
        wcat_h.append(np.concatenate([wq, wk, wv, wo], axis=1).astype(h16))

    per_core = []
    for c in range(N_CORES):
        b, h = divmod(c, GROUP)
        xq = np.ascontiguousarray(
            x[b].T.reshape(ET, P, T)[:, :, h * TQ:(h + 1) * TQ]).astype(h16)
        tm = np.ascontiguousarray(
            np.concatenate([tmcat[:, k * T + c * TE8: k * T + (c + 1) * TE8]
                            for k in range(3)], axis=1))
        wh = np.ascontiguousarray(wcat_h[h][:, b * WHALF:(b + 1) * WHALF])
        bias6 = np.stack([bq[h * KV + ct * P: h * KV + (ct + 1) * P]
                          for ct in range(GROUP)]
                         + [bk[h * D:(h + 1) * D], bv[h * D:(h + 1) * D]],
                         axis=1).astype(f)
        bo4 = np.ascontiguousarray(
            bo[h * KV:(h + 1) * KV].reshape(GROUP, P).T.astype(f))
        per_core.append({
            "xq": xq, "wh": wh, "tm": tm, "ptm": ptm, "ident": ident,
            "bias6": np.ascontiguousarray(bias6), "bo4": bo4,
        })
    return per_core


def kernel(**inputs):
    x = np.asarray(inputs["x"], np.float32)
    nc = _build_program()
    in_maps = _host_inputs(
        x, *(np.asarray(inputs[k], np.float32)
             for k in ("Wq", "bq", "Wk", "bk", "Wv", "bv", "Wo", "bo")))
    res = run_bass_kernel_spmd(nc, in_maps, list(range(N_CORES)))
    out = np.empty((B, T, E), np.float32)
    for b in range(B):
        rows = np.concatenate(
            [res.results[b * GROUP + h]["outt"].reshape(KV, T)
             for h in range(GROUP)], axis=0)                  # [E, T] fp16
        out[b] = rows.T.astype(np.float32)
    return out


# revision 26
# speedup vs baseline: 1.0578x; 1.0578x over previous
"""Grouped-Query Attention (B=2, T=2048, E=2048, 16 Q heads / 4 KV heads, RoPE,
causal) as a Bass/Tile kernel on 8 Trainium2 NeuronCores.

Sharding: core c = 4*b + h handles batch b (of 2) and KV-head group h (of 4,
i.e. 4 q-heads + 1 kv head).  Host->device traffic is minimized (it dominates
the end-to-end time): inputs ship in fp16 with no cross-core duplication and
are reconstructed on device with AllGather collectives:

  - x ships token-quartered (core h gets quarter h of x[b]) -> 4-core AllGather
  - weights ship halved across the batch pair (cores h and h+4 need the same
    head-h weights) -> 2-core AllGather
  - rope tables + causal mask ship 1/8th per core -> 8-core AllGather
  - attention outputs y are AllGathered within each batch group so every core
    computes a distinct 512-row slice of the final out-projection; the output
    is 4x smaller and needs no host-side reduction.

On device everything is channel-major ([channel, token]); matmuls contract
along the partition axis with 512-wide moving operands, fp16 operands with
fp32 PSUM accumulation.
"""

import os
import sys

import numpy as np

try:
    import concourse.bass as bass
except ModuleNotFoundError:  # fresh grading dir: point at the in-container repo
    for p in ("/opt/trn_rl_repo", "/root/.axon_site/_ro/trn_rl_repo"):
        if os.path.isdir(p) and p not in sys.path:
            sys.path.insert(0, p)
    import concourse.bass as bass

from contextlib import ExitStack

import concourse.tile as tile
from concourse import bacc, mybir
from concourse.bass_utils import run_bass_kernel_spmd

# ---- problem constants (hardcoded per contract) ----
B, T, E = 2, 2048, 2048
N_QHEAD, N_KVHEAD = 16, 4
GROUP = N_QHEAD // N_KVHEAD          # 4 q heads per kv head
D = E // N_QHEAD                     # 128 head dim
KV = E // GROUP                      # 512 kv dim
ROPE_BASE = 10000.0
N_CORES = 8

P = 128                              # partitions
ET = E // P                          # 16 e-tiles
TT = T // P                          # 16 token tiles
TC = 512                             # moving-dim chunk (max for fp32 PSUM)
NTC = T // TC                        # 4 token chunks
TQ = T // GROUP                      # 512-token x quarter per core
WCOLS = (ET * GROUP + ET + ET + GROUP * ET) * P   # wq|wk|wv|wo = 20480
WHALF = WCOLS // 2                   # 10240 (= wq|wk and wv|wo exactly)
TME = (3 * T) // N_CORES             # 768 table/mask eighth columns

F32 = mybir.dt.float32
F16 = mybir.dt.float16

G_BATCH = [[0, 1, 2, 3], [4, 5, 6, 7]]           # 4-core batch groups
G_PAIR = [[0, 4], [1, 5], [2, 6], [3, 7]]        # same-head pairs
G_ALL = [list(range(N_CORES))]

_CACHE = {}


def _build_program():
    """Build + compile the (SPMD-identical) Bass program once per process."""
    if "nc" in _CACHE:
        return _CACHE["nc"]

    nc = bacc.Bacc("TRN2", target_bir_lowering=False, debug=False,
                   num_devices=N_CORES)

    dram = {}
    def din(name, shape, dt=F16):
        dram[name] = nc.dram_tensor(name, list(shape), dt,
                                    kind="ExternalInput").ap()
    din("xq", (ET, P, TQ))          # token-quarter h of x[b].T, (e, p, t)
    din("wh", (P, WHALF))           # batch-half of [wq|wk|wv|wo] tiles
    din("thetas", (1, P), F32)      # rope inverse frequencies (row vector)
    din("bias6", (P, 6), F32)       # per-ctile biases: 4x bq, bk, bv
    din("bo4", (P, GROUP), F32)     # bo slice for this core's 4 j-tiles
    outt = nc.dram_tensor("outt", [GROUP, P, T], F16,
                          kind="ExternalOutput").ap()

    with tile.TileContext(nc) as tc:
        with ExitStack() as ctx, nc.allow_low_precision(
                reason="fp16 operands; accumulation stays fp32 in PSUM"):
            dpool = ctx.enter_context(tc.tile_pool(name="dram", bufs=1,
                                                   space="DRAM"))
            persist = ctx.enter_context(tc.tile_pool(name="persist", bufs=1))

            def ptile(shape, name, dt=F16):
                return persist.tile(shape, dt, tag=name, name=name)

            # ---------- DRAM bounce + gathered buffers ----------
            # CC can't read IO tensors -> bounce first.  The weight AllGather
            # goes first (cross-die pairs, D2D-bandwidth-bound); x is gathered
            # in two halves so phase 1 can start after the first half lands;
            # tables gather last (not needed until RoPE).
            TH = TQ // 2                              # 256-token half-quarters
            wb = dpool.tile([P, WHALF], F16)
            xb1 = dpool.tile([ET, P, TH], F16)
            xb2 = dpool.tile([ET, P, TH], F16)
            wg = dpool.tile([2, P, WHALF], F16)
            xg1 = dpool.tile([GROUP, ET, P, TH], F16)  # tokens m*512+[0,256)
            xg2 = dpool.tile([GROUP, ET, P, TH], F16)  # tokens m*512+[256,512)
            yb_g = [dpool.tile([P, T], F16, name=f"yb{g}")
                    for g in range(GROUP)]
            yg_g = [dpool.tile([GROUP, P, T], F16, name=f"yg{g}")
                    for g in range(GROUP)]

            nc.sync.dma_start(wb[:], dram["wh"][:])
            nc.sync.dma_start(xb1[:], dram["xq"][:, :, 0:TH])
            nc.sync.dma_start(xb2[:], dram["xq"][:, :, TH:TQ])
            nc.gpsimd.collective_compute(
                "AllGather", mybir.AluOpType.bypass, replica_groups=G_PAIR,
                ins=[wb.opt()], outs=[wg.opt()])
            nc.gpsimd.collective_compute(
                "AllGather", mybir.AluOpType.bypass, replica_groups=G_BATCH,
                ins=[xb1.opt()], outs=[xg1.opt()])
            nc.gpsimd.collective_compute(
                "AllGather", mybir.AluOpType.bypass, replica_groups=G_BATCH,
                ins=[xb2.opt()], outs=[xg2.opt()])

            # ---------- persistent SBUF tiles ----------
            wq_sb = ptile([P, ET * GROUP * P], "wq_sb")
            wk_sb = ptile([P, ET * P], "wk_sb")
            wv_sb = ptile([P, ET * P], "wv_sb")
            wo_sb = ptile([P, ET * GROUP * P], "wo_sb")
            bias6_sb = ptile([P, 8], "bias6_sb", F32)  # padded to 32B
            bo4_sb = ptile([P, GROUP], "bo4_sb", F32)
            thetas_sb = ptile([1, P], "thetas_sb", F32)
            ptm_sb = ptile([P, P], "ptm_sb")
            ptm_neg = ptile([P, P], "ptm_neg")
            ident_sb = ptile([P, P], "ident_sb")
            trow_i = ptile([1, T], "trow_i", mybir.dt.int32)
            trow_sb = ptile([1, T], "trow_sb", F32)
            qT_sb = ptile([P, GROUP * T], "qT_sb")    # 4 heads, channel-major
            kT_sb = ptile([P, T], "kT_sb")
            vT_sb = ptile([P, T], "vT_sb")
            vtok_sb = ptile([P, T], "vtok_sb")        # token-major v
            y_sb = ptile([P, GROUP * T], "y_sb")      # yT per head
            cos_sb = ptile([P, T], "cos_sb")
            sin_sb = ptile([P, T], "sin_sb")
            mask4_sb = ptile([P, GROUP * TC], "mask4_sb")
            ones1_sb = ptile([P, 8], "ones1_sb")
            onesr_sb = ptile([1, P], "onesr_sb")

            # pools (SBUF)
            xw = ctx.enter_context(tc.tile_pool(name="xw", bufs=2))    # x / y chunks
            ck = ctx.enter_context(tc.tile_pool(name="ck", bufs=2))    # exp tiles
            osb = ctx.enter_context(tc.tile_pool(name="osb", bufs=2))  # out staging
            # pools (PSUM): statically 4 + 4 = 8 banks
            pacc = ctx.enter_context(tc.tile_pool(name="pacc", bufs=4, space="PSUM"))
            pbig = ctx.enter_context(tc.tile_pool(name="pbig", bufs=2, space="PSUM"))

            # ---------- load + generate constants ----------
            # everything below runs in the shadow of the input AllGathers
            nc.sync.dma_start(bias6_sb[:, 0:6], dram["bias6"][:])
            nc.sync.dma_start(bo4_sb[:], dram["bo4"][:])
            nc.sync.dma_start(thetas_sb[:], dram["thetas"][:])
            nc.vector.memset(ones1_sb[:], 1.0)
            nc.vector.memset(onesr_sb[:], 1.0)

            # identity: 1 where col == p
            nc.gpsimd.memset(ident_sb[:], 1.0)
            nc.gpsimd.affine_select(
                ident_sb[:], ident_sb[:], compare_op=mybir.AluOpType.is_equal,
                fill=0.0, base=0, channel_multiplier=-1, pattern=[[1, P]])
            # rope rotation Pm^T: -1 at col==p-64 (p>=64), +1 at col==p+64
            nc.gpsimd.memset(ptm_sb[:], -1.0)
            nc.gpsimd.affine_select(
                ptm_sb[:], ptm_sb[:], compare_op=mybir.AluOpType.is_equal,
                fill=0.0, base=P // 2, channel_multiplier=-1, pattern=[[1, P]])
            nc.gpsimd.memset(ptm_neg[:], 1.0)
            nc.gpsimd.affine_select(
                ptm_neg[:], ptm_neg[:], compare_op=mybir.AluOpType.is_equal,
                fill=0.0, base=-(P // 2), channel_multiplier=-1,
                pattern=[[1, P]])
            nc.vector.tensor_add(ptm_sb[:], ptm_sb[:], ptm_neg[:])
            # causal masks (4 diagonal-straddle tiles): 1 where t' >= p+128r
            nc.gpsimd.memset(mask4_sb[:], 1.0)
            nc.gpsimd.affine_select(
                mask4_sb[:], mask4_sb[:], compare_op=mybir.AluOpType.is_ge,
                fill=0.0, base=0, channel_multiplier=-1,
                pattern=[[-P, GROUP], [1, TC]])
            # rope tables: ang[p,t] = thetas[p]*(t+1); sin/cos via range
            # reduction to [-pi, pi) and the ACT Sin LUT
            # no mod ALU op on TRN2 -> reduce via k = round(x/2pi) using the
            # round-to-nearest f32->i32 cast, r = x - 2pi*k in [-pi, pi]
            PI = float(np.pi)
            nc.gpsimd.iota(trow_i[:], pattern=[[1, T]], base=1,
                           channel_multiplier=0)
            nc.vector.tensor_copy(trow_sb[:], trow_i[:])
            for c in range(NTC):
                cs = slice(c * TC, (c + 1) * TC)
                aps = pacc.tile([P, TC], F32, tag="acc", name="aps")
                nc.tensor.matmul(aps[:], thetas_sb[:], trow_sb[:, cs],
                                 start=True, stop=True)
                for dst, shift in ((sin_sb, 0.0), (cos_sb, 0.5 * PI)):
                    sc = osb.tile([P, TC], F32, tag="ost", name="sc", bufs=4)
                    yi = osb.tile([P, TC], mybir.dt.int32, tag="yi",
                                  name="yi", bufs=1)
                    yf = osb.tile([P, TC], F32, tag="ost", name="yf", bufs=4)
                    nc.vector.tensor_scalar(sc[:], aps[:], shift,
                                            1.0 / (2 * PI),
                                            mybir.AluOpType.add,
                                            mybir.AluOpType.mult)
                    nc.vector.tensor_copy(yi[:], sc[:])
                    nc.vector.tensor_copy(yf[:], yi[:])
                    nc.vector.tensor_scalar(yf[:], yf[:], -2 * PI, shift,
                                            mybir.AluOpType.mult,
                                            mybir.AluOpType.add)
                    nc.vector.tensor_add(yf[:], aps[:], yf[:])
                    nc.scalar.activation(dst[:, cs], yf[:],
                                         mybir.ActivationFunctionType.Sin)
            # weights from the pair-gathered halves: [wq|wk] then [wv|wo]
            WQC = ET * GROUP * P                      # 8192
            for q4 in range(4):                       # split for DMA parallelism
                s = slice(q4 * WQC // 4, (q4 + 1) * WQC // 4)
                nc.sync.dma_start(wq_sb[:, s], wg[0][:, s])
            nc.sync.dma_start(wk_sb[:], wg[0][:, WQC:WQC + ET * P])
            nc.sync.dma_start(wv_sb[:], wg[1][:, 0:ET * P])
            for q4 in range(4):
                s = slice(q4 * WQC // 4, (q4 + 1) * WQC // 4)
                nc.sync.dma_start(wo_sb[:, s],
                                  wg[1][:, ET * P + q4 * WQC // 4:
                                        ET * P + (q4 + 1) * WQC // 4])
            # ---------- phase 1: QKV projections (channel-major) ----------
            # qT[c,t] = sum_e WqT[e,c] * xT[e,t]  (+bias at evacuation)
            XC = 256                      # x token-chunk width
            NXC = T // XC

            def proj_dst(ct):
                if ct < GROUP:
                    return qT_sb[:, ct * T:(ct + 1) * T]
                return (kT_sb if ct == GROUP else vT_sb)[:, :]

            for xc in [0, 2, 4, 6, 1, 3, 5, 7]:     # first-half chunks first
                mm = xc // 2
                x_sb = xw.tile([P, ET * XC], F16, tag="xw", name="x_sb")
                x3 = x_sb[:].rearrange("p (e t) -> p e t", e=ET)
                xgh = xg1 if xc % 2 == 0 else xg2
                xd = xgh[mm][:, :, :].rearrange("e p t -> p e t")
                for q4 in range(4):
                    nc.sync.dma_start(x3[:, q4 * 4:(q4 + 1) * 4, :],
                                      xd[:, q4 * 4:(q4 + 1) * 4, :])
                for half in range(2):          # <=3 live PSUM accums at a time
                    for ct3 in range(3):
                        ct = half * 3 + ct3
                        ppr = pacc.tile([P, XC], F32, tag="acc", name="ppr")
                        for e in range(ET):
                            if ct < GROUP:
                                lhs = wq_sb[:, (e * GROUP + ct) * P:
                                            (e * GROUP + ct + 1) * P]
                            elif ct == GROUP:
                                lhs = wk_sb[:, e * P:(e + 1) * P]
                            else:
                                lhs = wv_sb[:, e * P:(e + 1) * P]
                            nc.tensor.matmul(
                                ppr[:], lhs,
                                x_sb[:, e * XC:(e + 1) * XC],
                                start=(e == 0), stop=(e == ET - 1))
                        dst = proj_dst(ct)
                        nc.vector.tensor_scalar_add(
                            dst[:, xc * XC:(xc + 1) * XC], ppr[:],
                            bias6_sb[:, ct:ct + 1])

            # ---------- phase 1b: RoPE (shared tables; k scaled after) ------
            def rope(dst_full):
                for c in range(NTC):
                    cs = slice(c * TC, (c + 1) * TC)
                    rot_ps = pacc.tile([P, TC], F32, tag="acc", name="rot_ps")
                    nc.tensor.matmul(rot_ps[:], ptm_sb[:], dst_full[:, cs],
                                     start=True, stop=True)
                    tmp = osb.tile([P, TC], F32, tag="ost", name="tmp", bufs=4)
                    nc.vector.tensor_mul(tmp[:], rot_ps[:], sin_sb[:, cs])
                    nc.vector.tensor_mul(dst_full[:, cs],
                                         dst_full[:, cs], cos_sb[:, cs])
                    nc.vector.tensor_add(dst_full[:, cs],
                                         dst_full[:, cs], tmp[:])

            rope(kT_sb[:, :])
            # fold the 1/sqrt(D) score scale into k
            nc.vector.tensor_scalar_mul(kT_sb[:], kT_sb[:],
                                        float(1.0 / np.sqrt(D)))

            # ---------- phase 1c: v -> token-major via PE transpose ----------
            for j in range(TT):
                vps = pacc.tile([P, P], F16, tag="acc", name="vps")
                nc.tensor.transpose(vps[:], vT_sb[:, j * P:(j + 1) * P],
                                    ident_sb[:])
                nc.vector.tensor_copy(vtok_sb[:, j * P:(j + 1) * P], vps[:])

            # ---------- phase 2: causal attention per (head, tq-chunk) -------
            # transposed scores: sT[tk, tq] = kT_j^T . qT ; softmax over tk via
            # ones-matmul column sums; normalization folded in at the end.
            # rope of head h+1 (DVE-heavy) overlaps attention of head h
            # (PE-heavy) -- emitted just-in-time per head.
            for h in range(GROUP):
                rope(qT_sb[:, h * T:(h + 1) * T])
                for qc in range(NTC):
                    jmax = GROUP * qc + GROUP - 1
                    ng2 = 2 * (qc + 1)          # groups of 2 j-tiles
                    yps = pacc.tile([P, TC], F32, tag="acc", name="yps")
                    sps = pacc.tile([1, TC], F32, tag="acc", name="sps")

                    def scores(g):
                        # one [128,1024] PSUM tile holding 2 j-tiles' scores
                        spsum = pbig.tile([P, 2 * TC], F32, tag="big",
                                          name="spsum")
                        for sub in range(2):
                            j = 2 * g + sub
                            nc.tensor.matmul(
                                spsum[:, sub * TC:(sub + 1) * TC],
                                kT_sb[:, j * P:(j + 1) * P],
                                qT_sb[:, h * T + qc * TC:
                                      h * T + (qc + 1) * TC],
                                start=True, stop=True)
                        return spsum

                    # software-pipelined: scores of group g+1 are emitted
                    # before the exp/AV consumers of group g so the PE never
                    # sits behind the ACT exp in program order.  The softmax
                    # denominator is accumulated on the DVE (partial column
                    # sums over j-tiles) so only one ones-matmul per q-chunk
                    # runs on the PE instead of one per j-tile.
                    eacc = ck.tile([P, TC], F16, tag="eacc", name="eacc")
                    spsum = scores(0)
                    for g in range(ng2):
                        nxt = scores(g + 1) if g + 1 < ng2 else None
                        eg = ck.tile([P, 2 * TC], F16, tag="ck", name="eg",
                                     bufs=3)
                        nc.scalar.activation(eg[:], spsum[:],
                                             mybir.ActivationFunctionType.Exp)
                        if g >= ng2 - 2:        # diagonal-straddling groups
                            half = g - (ng2 - 2)
                            nc.vector.tensor_mul(
                                eg[:], eg[:],
                                mask4_sb[:, half * 2 * TC:(half + 1) * 2 * TC])
                        if g == 0:
                            nc.vector.tensor_add(eacc[:], eg[:, 0:TC],
                                                 eg[:, TC:2 * TC])
                        else:
                            nc.vector.tensor_add(eacc[:], eacc[:],
                                                 eg[:, 0:TC])
                            nc.vector.tensor_add(eacc[:], eacc[:],
                                                 eg[:, TC:2 * TC])
                        for sub in range(2):
                            j = 2 * g + sub
                            nc.tensor.matmul(
                                yps[:], vtok_sb[:, j * P:(j + 1) * P],
                                eg[:, sub * TC:(sub + 1) * TC],
                                start=(j == 0), stop=(j == jmax))
                        spsum = nxt
                    nc.tensor.matmul(sps[:], ones1_sb[:, 0:1], eacc[:],
                                     start=True, stop=True)
                    # normalize: y /= colsum (broadcast 1/sum via K=1 matmul)
                    rec = osb.tile([1, TC], F16, tag="rec", name="rec", bufs=1)
                    nc.vector.reciprocal(rec[:], sps[:])
                    bps = pacc.tile([P, TC], F32, tag="acc", name="bps")
                    nc.tensor.matmul(bps[:], onesr_sb[:], rec[:],
                                     start=True, stop=True)
                    bcs = osb.tile([P, TC], F32, tag="bc", name="bcs", bufs=1)
                    nc.scalar.copy(bcs[:], bps[:])
                    nc.vector.tensor_mul(
                        y_sb[:, h * T + qc * TC: h * T + (qc + 1) * TC],
                        yps[:], bcs[:])
                # stage + gather this head's y now: the CC overlaps the next
                # head's attention; only head 3's gather sits on the tail
                nc.sync.dma_start(yb_g[h][:], y_sb[:, h * T:(h + 1) * T])
                nc.gpsimd.collective_compute(
                    "AllGather", mybir.AluOpType.bypass,
                    replica_groups=G_BATCH,
                    ins=[yb_g[h].opt()], outs=[yg_g[h].opt()])

            # ---------- phase 3: out-projection rows [h*512,(h+1)*512) ------
            # outT[j,t] = sum_c WoS[j,c] * yT[c,t] + bo[j]; y gathered from
            # all 4 cores of this batch group, streamed by token chunk.
            for c in range(NTC):
                ysb = xw.tile([P, ET * TC], F16, tag="ysb", name="ysb",
                              bufs=3)
                y4 = ysb[:].rearrange("p (h g t) -> p h g t", h=GROUP, g=GROUP)
                for g in range(GROUP):
                    src = yg_g[g][:, :, c * TC:(c + 1) * TC].rearrange(
                        "h d t -> d h t")
                    nc.sync.dma_start(y4[:, :, g, :], src)
                for jj in range(GROUP):
                    ops = pacc.tile([P, TC], F32, tag="acc", name="ops")
                    for ct in range(ET):
                        nc.tensor.matmul(
                            ops[:], wo_sb[:, (ct * GROUP + jj) * P:
                                          (ct * GROUP + jj + 1) * P],
                            ysb[:, ct * TC:(ct + 1) * TC],
                            start=(ct == 0), stop=(ct == ET - 1))
                    ost = osb.tile([P, TC], F16, tag="ost2", name="ost", bufs=4)
                    nc.vector.tensor_scalar_add(ost[:], ops[:],
                                                bo4_sb[:, jj:jj + 1])
                    nc.sync.dma_start(outt[jj][:, c * TC:(c + 1) * TC], ost[:])

    nc.compile()
    _CACHE["nc"] = nc
    return nc


def _host_inputs(x, Wq, bq, Wk, bk, Wv, bv, Wo, bo):
    """Per-core input dicts (fp16 payloads, layouts matching the DRAM decls)."""
    f = np.float32
    h16 = np.float16
    i = np.arange(D // 2, dtype=np.float64)
    th_half = ROPE_BASE ** (-2.0 * i / D)
    thetas = np.concatenate([th_half, th_half]).astype(f).reshape(1, P)

    # per-head weight blocks [wq|wk|wv|wo] -> [P, 20480] fp16
    wcat_h = []
    for h in range(GROUP):
        WqS = Wq[h * KV:(h + 1) * KV, :]                                # [512, E]
        wq = WqS.T.reshape(ET, P, GROUP, P).transpose(1, 0, 2, 3).reshape(P, -1)
        WkS = Wk[h * D:(h + 1) * D, :]
        wk = WkS.T.reshape(ET, P, P).transpose(1, 0, 2).reshape(P, -1)
        WvS = Wv[h * D:(h + 1) * D, :]
        wv = WvS.T.reshape(ET, P, P).transpose(1, 0, 2).reshape(P, -1)
        WoS = Wo[h * KV:(h + 1) * KV, :]                                # [512, E]
        wo = WoS.reshape(GROUP, P, ET, P).transpose(3, 2, 0, 1).reshape(P, -1)
        wcat_h.append(np.concatenate([wq, wk, wv, wo], axis=1).astype(h16))

    per_core = []
    for c in range(N_CORES):
        b, h = divmod(c, GROUP)
        xq = np.ascontiguousarray(
            x[b].T.reshape(ET, P, T)[:, :, h * TQ:(h + 1) * TQ]).astype(h16)
        wh = np.ascontiguousarray(wcat_h[h][:, b * WHALF:(b + 1) * WHALF])
        bias6 = np.stack([bq[h * KV + ct * P: h * KV + (ct + 1) * P]
                          for ct in range(GROUP)]
                         + [bk[h * D:(h + 1) * D], bv[h * D:(h + 1) * D]],
                         axis=1).astype(f)
        bo4 = np.ascontiguousarray(
            bo[h * KV:(h + 1) * KV].reshape(GROUP, P).T.astype(f))
        per_core.append({
            "xq": xq, "wh": wh, "thetas": thetas,
            "bias6": np.ascontiguousarray(bias6), "bo4": bo4,
        })
    return per_core


def kernel(**inputs):
    x = np.asarray(inputs["x"], np.float32)
    nc = _build_program()
    in_maps = _host_inputs(
        x, *(np.asarray(inputs[k], np.float32)
             for k in ("Wq", "bq", "Wk", "bk", "Wv", "bv", "Wo", "bo")))
    res = run_bass_kernel_spmd(nc, in_maps, list(range(N_CORES)))
    out = np.empty((B, T, E), np.float32)
    for b in range(B):
        rows = np.concatenate(
            [res.results[b * GROUP + h]["outt"].reshape(KV, T)
             for h in range(GROUP)], axis=0)                  # [E, T] fp16
        out[b] = rows.T.astype(np.float32)
    return out


# revision 31
# speedup vs baseline: 1.1047x; 1.0443x over previous
"""Grouped-Query Attention (B=2, T=2048, E=2048, 16 Q heads / 4 KV heads, RoPE,
causal) as a Bass/Tile kernel on 8 Trainium2 NeuronCores.

Sharding: core c = 4*b + h handles batch b (of 2) and KV-head group h (of 4,
i.e. 4 q-heads + 1 kv head).  Host->device traffic is minimized (it dominates
the end-to-end time): inputs ship in fp16 with no cross-core duplication and
are reconstructed on device with AllGather collectives:

  - x ships token-quartered (core h gets quarter h of x[b]) -> 4-core AllGather
  - weights ship halved across the batch pair (cores h and h+4 need the same
    head-h weights) -> 2-core AllGather
  - rope tables + causal mask ship 1/8th per core -> 8-core AllGather
  - attention outputs y are AllGathered within each batch group so every core
    computes a distinct 512-row slice of the final out-projection; the output
    is 4x smaller and needs no host-side reduction.

On device everything is channel-major ([channel, token]); matmuls contract
along the partition axis with 512-wide moving operands, fp16 operands with
fp32 PSUM accumulation.
"""

import os
import sys

import numpy as np

try:
    import concourse.bass as bass
except ModuleNotFoundError:  # fresh grading dir: point at the in-container repo
    for p in ("/opt/trn_rl_repo", "/root/.axon_site/_ro/trn_rl_repo"):
        if os.path.isdir(p) and p not in sys.path:
            sys.path.insert(0, p)
    import concourse.bass as bass

from contextlib import ExitStack

import concourse.tile as tile
from concourse import bacc, mybir
from concourse.bass_utils import run_bass_kernel_spmd

# ---- problem constants (hardcoded per contract) ----
B, T, E = 2, 2048, 2048
N_QHEAD, N_KVHEAD = 16, 4
GROUP = N_QHEAD // N_KVHEAD          # 4 q heads per kv head
D = E // N_QHEAD                     # 128 head dim
KV = E // GROUP                      # 512 kv dim
ROPE_BASE = 10000.0
N_CORES = 8

P = 128                              # partitions
ET = E // P                          # 16 e-tiles
TT = T // P                          # 16 token tiles
TC = 512                             # moving-dim chunk (max for fp32 PSUM)
NTC = T // TC                        # 4 token chunks
TQ = T // GROUP                      # 512-token x quarter per core
WCOLS = (ET * GROUP + ET + ET + GROUP * ET) * P   # wq|wk|wv|wo = 20480
WHALF = WCOLS // 2                   # 10240 (= wq|wk and wv|wo exactly)
TME = (3 * T) // N_CORES             # 768 table/mask eighth columns

F32 = mybir.dt.float32
F16 = mybir.dt.float16

G_BATCH = [[0, 1, 2, 3], [4, 5, 6, 7]]           # 4-core batch groups
G_PAIR = [[0, 4], [1, 5], [2, 6], [3, 7]]        # same-head pairs
G_ALL = [list(range(N_CORES))]

_CACHE = {}


def _build_program():
    """Build + compile the (SPMD-identical) Bass program once per process."""
    if "nc" in _CACHE:
        return _CACHE["nc"]

    nc = bacc.Bacc("TRN2", target_bir_lowering=False, debug=False,
                   num_devices=N_CORES)

    dram = {}
    def din(name, shape, dt=F16):
        dram[name] = nc.dram_tensor(name, list(shape), dt,
                                    kind="ExternalInput").ap()
    WQKV = (ET * GROUP + 2 * ET) * P // 2   # 6144: half of [wq|wk|wv]
    WOH = GROUP * ET * P // 2               # 4096: half of wo
    din("xq", (ET, P, TQ))          # token-quarter h of x[b].T, (e, p, t)
    din("whq", (P, WQKV))           # batch-half of [wq|wk|wv] tiles
    din("who", (P, WOH))            # batch-half of wo tiles
    din("thetas", (1, P), F32)      # rope inverse frequencies (row vector)
    din("bias6", (P, 6), F32)       # per-ctile biases: 4x bq, bk, bv
    din("bo4", (P, GROUP), F32)     # bo slice for this core's 4 j-tiles
    outt = nc.dram_tensor("outt", [GROUP, P, T], F16,
                          kind="ExternalOutput").ap()

    with tile.TileContext(nc) as tc:
        with ExitStack() as ctx, nc.allow_low_precision(
                reason="fp16 operands; accumulation stays fp32 in PSUM"):
            dpool = ctx.enter_context(tc.tile_pool(name="dram", bufs=1,
                                                   space="DRAM"))
            persist = ctx.enter_context(tc.tile_pool(name="persist", bufs=1))

            def ptile(shape, name, dt=F16):
                return persist.tile(shape, dt, tag=name, name=name)

            # ---------- DRAM bounce + gathered buffers ----------
            # CC can't read IO tensors -> bounce first.  The weight AllGather
            # goes first (cross-die pairs, D2D-bandwidth-bound); x is gathered
            # in two halves so phase 1 can start after the first half lands;
            # tables gather last (not needed until RoPE).
            NXS = 4                                   # x gathered in 4 slices
            TH = TQ // NXS                            # 128-token slices
            wqb = dpool.tile([P, WQKV], F16)
            wob = dpool.tile([P, WOH], F16)
            xb_s = [dpool.tile([ET, P, TH], F16, name=f"xb{i}")
                    for i in range(NXS)]
            wgq = dpool.tile([2, P, WQKV], F16)
            wgo = dpool.tile([2, P, WOH], F16)
            xg_s = [dpool.tile([GROUP, ET, P, TH], F16, name=f"xg{i}")
                    for i in range(NXS)]              # tokens m*512+[128i,..)
            yb_g = [dpool.tile([P, T], F16, name=f"yb{g}")
                    for g in range(GROUP)]
            yg_g = [dpool.tile([GROUP, P, T], F16, name=f"yg{g}")
                    for g in range(GROUP)]

            nc.sync.dma_start(wqb[:], dram["whq"][:])
            for i in range(NXS):
                nc.sync.dma_start(xb_s[i][:],
                                  dram["xq"][:, :, i * TH:(i + 1) * TH])
            nc.sync.dma_start(wob[:], dram["who"][:])
            # CC order = consumption order: qkv weights, x slices, wo last
            nc.gpsimd.collective_compute(
                "AllGather", mybir.AluOpType.bypass, replica_groups=G_PAIR,
                ins=[wqb.opt()], outs=[wgq.opt()])
            for i in range(NXS):
                nc.gpsimd.collective_compute(
                    "AllGather", mybir.AluOpType.bypass,
                    replica_groups=G_BATCH,
                    ins=[xb_s[i].opt()], outs=[xg_s[i].opt()])
            nc.gpsimd.collective_compute(
                "AllGather", mybir.AluOpType.bypass, replica_groups=G_PAIR,
                ins=[wob.opt()], outs=[wgo.opt()])

            # ---------- persistent SBUF tiles ----------
            wq_sb = ptile([P, ET * GROUP * P], "wq_sb")
            wk_sb = ptile([P, ET * P], "wk_sb")
            wv_sb = ptile([P, ET * P], "wv_sb")
            wo_sb = ptile([P, ET * GROUP * P], "wo_sb")
            bias6_sb = ptile([P, 8], "bias6_sb", F32)  # padded to 32B
            bo4_sb = ptile([P, GROUP], "bo4_sb", F32)
            thetas_sb = ptile([1, P], "thetas_sb", F32)
            ptm_sb = ptile([P, P], "ptm_sb")
            ptm_neg = ptile([P, P], "ptm_neg")
            ident_sb = ptile([P, P], "ident_sb")
            trow_i = ptile([1, T], "trow_i", mybir.dt.int32)
            trow_sb = ptile([1, T], "trow_sb", F32)
            qT_sb = ptile([P, GROUP * T], "qT_sb")    # 4 heads, channel-major
            kT_sb = ptile([P, T], "kT_sb")
            vT_sb = ptile([P, T], "vT_sb")
            vtok_sb = ptile([P, T], "vtok_sb")        # token-major v
            y_sb = ptile([P, GROUP * T], "y_sb")      # yT per head
            cos_sb = ptile([P, T], "cos_sb")
            sin_sb = ptile([P, T], "sin_sb")
            mask4_sb = ptile([P, GROUP * TC], "mask4_sb")
            ones1_sb = ptile([P, 8], "ones1_sb")
            onesr_sb = ptile([1, P], "onesr_sb")

            # pools (SBUF)
            xw = ctx.enter_context(tc.tile_pool(name="xw", bufs=2))    # x / y chunks
            ck = ctx.enter_context(tc.tile_pool(name="ck", bufs=2))    # exp tiles
            osb = ctx.enter_context(tc.tile_pool(name="osb", bufs=2))  # out staging
            # pools (PSUM): statically 4 + 4 = 8 banks
            pacc = ctx.enter_context(tc.tile_pool(name="pacc", bufs=4, space="PSUM"))
            pbig = ctx.enter_context(tc.tile_pool(name="pbig", bufs=2, space="PSUM"))

            # ---------- load + generate constants ----------
            # everything below runs in the shadow of the input AllGathers
            nc.sync.dma_start(bias6_sb[:, 0:6], dram["bias6"][:])
            nc.sync.dma_start(bo4_sb[:], dram["bo4"][:])
            nc.sync.dma_start(thetas_sb[:], dram["thetas"][:])
            nc.vector.memset(ones1_sb[:], 1.0)
            nc.vector.memset(onesr_sb[:], 1.0)

            # identity: 1 where col == p
            nc.gpsimd.memset(ident_sb[:], 1.0)
            nc.gpsimd.affine_select(
                ident_sb[:], ident_sb[:], compare_op=mybir.AluOpType.is_equal,
                fill=0.0, base=0, channel_multiplier=-1, pattern=[[1, P]])
            # rope rotation Pm^T: -1 at col==p-64 (p>=64), +1 at col==p+64
            nc.gpsimd.memset(ptm_sb[:], -1.0)
            nc.gpsimd.affine_select(
                ptm_sb[:], ptm_sb[:], compare_op=mybir.AluOpType.is_equal,
                fill=0.0, base=P // 2, channel_multiplier=-1, pattern=[[1, P]])
            nc.gpsimd.memset(ptm_neg[:], 1.0)
            nc.gpsimd.affine_select(
                ptm_neg[:], ptm_neg[:], compare_op=mybir.AluOpType.is_equal,
                fill=0.0, base=-(P // 2), channel_multiplier=-1,
                pattern=[[1, P]])
            nc.vector.tensor_add(ptm_sb[:], ptm_sb[:], ptm_neg[:])
            # causal masks (4 diagonal-straddle tiles): 1 where t' >= p+128r
            nc.gpsimd.memset(mask4_sb[:], 1.0)
            nc.gpsimd.affine_select(
                mask4_sb[:], mask4_sb[:], compare_op=mybir.AluOpType.is_ge,
                fill=0.0, base=0, channel_multiplier=-1,
                pattern=[[-P, GROUP], [1, TC]])
            # rope tables: ang[p,t] = thetas[p]*(t+1); sin/cos via range
            # reduction to [-pi, pi) and the ACT Sin LUT
            # no mod ALU op on TRN2 -> reduce via k = round(x/2pi) using the
            # round-to-nearest f32->i32 cast, r = x - 2pi*k in [-pi, pi]
            PI = float(np.pi)
            nc.gpsimd.iota(trow_i[:], pattern=[[1, T]], base=1,
                           channel_multiplier=0)
            nc.vector.tensor_copy(trow_sb[:], trow_i[:])
            for c in range(NTC):
                cs = slice(c * TC, (c + 1) * TC)
                aps = pacc.tile([P, TC], F32, tag="acc", name="aps")
                nc.tensor.matmul(aps[:], thetas_sb[:], trow_sb[:, cs],
                                 start=True, stop=True)
                for dst, shift in ((sin_sb, 0.0), (cos_sb, 0.5 * PI)):
                    sc = osb.tile([P, TC], F32, tag="ost", name="sc", bufs=4)
                    yi = osb.tile([P, TC], mybir.dt.int32, tag="yi",
                                  name="yi", bufs=1)
                    yf = osb.tile([P, TC], F32, tag="ost", name="yf", bufs=4)
                    nc.vector.tensor_scalar(sc[:], aps[:], shift,
                                            1.0 / (2 * PI),
                                            mybir.AluOpType.add,
                                            mybir.AluOpType.mult)
                    nc.vector.tensor_copy(yi[:], sc[:])
                    nc.vector.tensor_copy(yf[:], yi[:])
                    nc.vector.tensor_scalar(yf[:], yf[:], -2 * PI, shift,
                                            mybir.AluOpType.mult,
                                            mybir.AluOpType.add)
                    nc.vector.tensor_add(yf[:], aps[:], yf[:])
                    nc.scalar.activation(dst[:, cs], yf[:],
                                         mybir.ActivationFunctionType.Sin)
            # weights from the pair-gathered halves:
            # wgq member 0 = wq tiles [0,48); member 1 = wq[48,64) | wk | wv
            WQC = ET * GROUP * P                      # 8192
            for q3 in range(3):                       # split for DMA parallelism
                s = slice(q3 * WQKV // 3, (q3 + 1) * WQKV // 3)
                nc.sync.dma_start(wq_sb[:, s], wgq[0][:, s])
            nc.sync.dma_start(wq_sb[:, WQKV:WQC], wgq[1][:, 0:WQC - WQKV])
            nc.sync.dma_start(wk_sb[:], wgq[1][:, WQC - WQKV:
                                               WQC - WQKV + ET * P])
            nc.sync.dma_start(wv_sb[:], wgq[1][:, WQC - WQKV + ET * P:WQKV])
            for q4 in range(4):
                s = slice(q4 * WOH // 2, (q4 + 1) * WOH // 2)
                m, off = divmod(q4, 2)
                nc.sync.dma_start(
                    wo_sb[:, s], wgo[m][:, off * WOH // 2:(off + 1) * WOH // 2])
            # ---------- phase 1: QKV projections (channel-major) ----------
            # qT[c,t] = sum_e WqT[e,c] * xT[e,t]  (+bias at evacuation)
            # chunks ordered by gather slice so compute starts on slice 0
            XC = TH                       # 128-token chunks (= x CC slices)

            def proj_dst(ct):
                if ct < GROUP:
                    return qT_sb[:, ct * T:(ct + 1) * T]
                return (kT_sb if ct == GROUP else vT_sb)[:, :]

            for i in range(NXS):
                for mm in range(GROUP):
                    off = mm * TQ + i * TH          # global token offset
                    x_sb = xw.tile([P, ET * XC], F16, tag="xw", name="x_sb")
                    x3 = x_sb[:].rearrange("p (e t) -> p e t", e=ET)
                    xd = xg_s[i][mm][:, :, :].rearrange("e p t -> p e t")
                    for q4 in range(4):
                        nc.sync.dma_start(x3[:, q4 * 4:(q4 + 1) * 4, :],
                                          xd[:, q4 * 4:(q4 + 1) * 4, :])
                    for half in range(2):      # <=3 live PSUM accums at a time
                        for ct3 in range(3):
                            ct = half * 3 + ct3
                            ppr = pacc.tile([P, XC], F32, tag="acc",
                                            name="ppr")
                            for e in range(ET):
                                if ct < GROUP:
                                    lhs = wq_sb[:, (e * GROUP + ct) * P:
                                                (e * GROUP + ct + 1) * P]
                                elif ct == GROUP:
                                    lhs = wk_sb[:, e * P:(e + 1) * P]
                                else:
                                    lhs = wv_sb[:, e * P:(e + 1) * P]
                                nc.tensor.matmul(
                                    ppr[:], lhs,
                                    x_sb[:, e * XC:(e + 1) * XC],
                                    start=(e == 0), stop=(e == ET - 1))
                            dst = proj_dst(ct)
                            nc.vector.tensor_scalar_add(
                                dst[:, off:off + XC], ppr[:],
                                bias6_sb[:, ct:ct + 1])

            # ---------- phase 1b: RoPE (shared tables; k scaled after) ------
            def rope(dst_full):
                for c in range(NTC):
                    cs = slice(c * TC, (c + 1) * TC)
                    rot_ps = pacc.tile([P, TC], F32, tag="acc", name="rot_ps")
                    nc.tensor.matmul(rot_ps[:], ptm_sb[:], dst_full[:, cs],
                                     start=True, stop=True)
                    tmp = osb.tile([P, TC], F32, tag="ost", name="tmp", bufs=4)
                    nc.vector.tensor_mul(tmp[:], rot_ps[:], sin_sb[:, cs])
                    nc.vector.tensor_mul(dst_full[:, cs],
                                         dst_full[:, cs], cos_sb[:, cs])
                    nc.vector.tensor_add(dst_full[:, cs],
                                         dst_full[:, cs], tmp[:])

            rope(kT_sb[:, :])
            # fold the 1/sqrt(D) score scale into k
            nc.vector.tensor_scalar_mul(kT_sb[:], kT_sb[:],
                                        float(1.0 / np.sqrt(D)))

            # ---------- phase 1c: v -> token-major via PE transpose ----------
            for j in range(TT):
                vps = pacc.tile([P, P], F16, tag="acc", name="vps")
                nc.tensor.transpose(vps[:], vT_sb[:, j * P:(j + 1) * P],
                                    ident_sb[:])
                nc.vector.tensor_copy(vtok_sb[:, j * P:(j + 1) * P], vps[:])

            # ---------- phase 2: causal attention per (head, tq-chunk) -------
            # transposed scores: sT[tk, tq] = kT_j^T . qT ; softmax over tk via
            # ones-matmul column sums; normalization folded in at the end.
            # rope of head h+1 (DVE-heavy) overlaps attention of head h
            # (PE-heavy) -- emitted just-in-time per head.
            for h in range(GROUP):
                rope(qT_sb[:, h * T:(h + 1) * T])
                for qc in range(NTC):
                    jmax = GROUP * qc + GROUP - 1
                    ng2 = 2 * (qc + 1)          # groups of 2 j-tiles
                    yps = pacc.tile([P, TC], F32, tag="acc", name="yps")
                    sps = pacc.tile([1, TC], F32, tag="acc", name="sps")

                    def scores(g):
                        # one [128,1024] PSUM tile holding 2 j-tiles' scores
                        spsum = pbig.tile([P, 2 * TC], F32, tag="big",
                                          name="spsum")
                        for sub in range(2):
                            j = 2 * g + sub
                            nc.tensor.matmul(
                                spsum[:, sub * TC:(sub + 1) * TC],
                                kT_sb[:, j * P:(j + 1) * P],
                                qT_sb[:, h * T + qc * TC:
                                      h * T + (qc + 1) * TC],
                                start=True, stop=True)
                        return spsum

                    # software-pipelined: scores of group g+1 are emitted
                    # before the exp/AV consumers of group g so the PE never
                    # sits behind the ACT exp in program order.  The softmax
                    # denominator is accumulated on the DVE (partial column
                    # sums over j-tiles) so only one ones-matmul per q-chunk
                    # runs on the PE instead of one per j-tile.
                    eacc = ck.tile([P, TC], F16, tag="eacc", name="eacc")
                    spsum = scores(0)
                    for g in range(ng2):
                        nxt = scores(g + 1) if g + 1 < ng2 else None
                        eg = ck.tile([P, 2 * TC], F16, tag="ck", name="eg",
                                     bufs=3)
                        nc.scalar.activation(eg[:], spsum[:],
                                             mybir.ActivationFunctionType.Exp)
                        if g >= ng2 - 2:        # diagonal-straddling groups
                            half = g - (ng2 - 2)
                            nc.vector.tensor_mul(
                                eg[:], eg[:],
                                mask4_sb[:, half * 2 * TC:(half + 1) * 2 * TC])
                        if g == 0:
                            nc.gpsimd.tensor_add(eacc[:], eg[:, 0:TC],
                                                 eg[:, TC:2 * TC])
                        else:
                            nc.gpsimd.tensor_add(eacc[:], eacc[:],
                                                 eg[:, 0:TC])
                            nc.gpsimd.tensor_add(eacc[:], eacc[:],
                                                 eg[:, TC:2 * TC])
                        for sub in range(2):
                            j = 2 * g + sub
                            nc.tensor.matmul(
                                yps[:], vtok_sb[:, j * P:(j + 1) * P],
                                eg[:, sub * TC:(sub + 1) * TC],
                                start=(j == 0), stop=(j == jmax))
                        spsum = nxt
                    nc.tensor.matmul(sps[:], ones1_sb[:, 0:1], eacc[:],
                                     start=True, stop=True)
                    # normalize: y /= colsum (broadcast 1/sum via K=1 matmul)
                    rec = osb.tile([1, TC], F16, tag="rec", name="rec", bufs=1)
                    nc.vector.reciprocal(rec[:], sps[:])
                    bps = pacc.tile([P, TC], F32, tag="acc", name="bps")
                    nc.tensor.matmul(bps[:], onesr_sb[:], rec[:],
                                     start=True, stop=True)
                    bcs = osb.tile([P, TC], F32, tag="bc", name="bcs", bufs=1)
                    nc.scalar.copy(bcs[:], bps[:])
                    nc.vector.tensor_mul(
                        y_sb[:, h * T + qc * TC: h * T + (qc + 1) * TC],
                        yps[:], bcs[:])
                # stage + gather this head's y now: the CC overlaps the next
                # head's attention; only head 3's gather sits on the tail
                nc.sync.dma_start(yb_g[h][:], y_sb[:, h * T:(h + 1) * T])
                nc.gpsimd.collective_compute(
                    "AllGather", mybir.AluOpType.bypass,
                    replica_groups=G_BATCH,
                    ins=[yb_g[h].opt()], outs=[yg_g[h].opt()])

            # ---------- phase 3: out-projection rows [h*512,(h+1)*512) ------
            # outT[j,t] = sum_c WoS[j,c] * yT[c,t] + bo[j]; y gathered from
            # all 4 cores of this batch group, streamed by token chunk.
            for c in range(NTC):
                ysb = xw.tile([P, ET * TC], F16, tag="ysb", name="ysb",
                              bufs=3)
                y4 = ysb[:].rearrange("p (h g t) -> p h g t", h=GROUP, g=GROUP)
                for g in range(GROUP):
                    src = yg_g[g][:, :, c * TC:(c + 1) * TC].rearrange(
                        "h d t -> d h t")
                    nc.sync.dma_start(y4[:, :, g, :], src)
                for jj in range(GROUP):
                    ops = pacc.tile([P, TC], F32, tag="acc", name="ops")
                    for ct in range(ET):
                        nc.tensor.matmul(
                            ops[:], wo_sb[:, (ct * GROUP + jj) * P:
                                          (ct * GROUP + jj + 1) * P],
                            ysb[:, ct * TC:(ct + 1) * TC],
                            start=(ct == 0), stop=(ct == ET - 1))
                    ost = osb.tile([P, TC], F16, tag="ost2", name="ost", bufs=4)
                    nc.vector.tensor_scalar_add(ost[:], ops[:],
                                                bo4_sb[:, jj:jj + 1])
                    nc.sync.dma_start(outt[jj][:, c * TC:(c + 1) * TC], ost[:])

    nc.compile()
    _CACHE["nc"] = nc
    return nc


def _host_inputs(x, Wq, bq, Wk, bk, Wv, bv, Wo, bo):
    """Per-core input dicts (fp16 payloads, layouts matching the DRAM decls)."""
    f = np.float32
    h16 = np.float16
    i = np.arange(D // 2, dtype=np.float64)
    th_half = ROPE_BASE ** (-2.0 * i / D)
    thetas = np.concatenate([th_half, th_half]).astype(f).reshape(1, P)

    # per-head weight blocks [wq|wk|wv|wo] -> [P, 20480] fp16
    wcat_h = []
    for h in range(GROUP):
        WqS = Wq[h * KV:(h + 1) * KV, :]                                # [512, E]
        wq = WqS.T.reshape(ET, P, GROUP, P).transpose(1, 0, 2, 3).reshape(P, -1)
        WkS = Wk[h * D:(h + 1) * D, :]
        wk = WkS.T.reshape(ET, P, P).transpose(1, 0, 2).reshape(P, -1)
        WvS = Wv[h * D:(h + 1) * D, :]
        wv = WvS.T.reshape(ET, P, P).transpose(1, 0, 2).reshape(P, -1)
        WoS = Wo[h * KV:(h + 1) * KV, :]                                # [512, E]
        wo = WoS.reshape(GROUP, P, ET, P).transpose(3, 2, 0, 1).reshape(P, -1)
        wcat_h.append(np.concatenate([wq, wk, wv, wo], axis=1).astype(h16))

    per_core = []
    for c in range(N_CORES):
        b, h = divmod(c, GROUP)
        xq = np.ascontiguousarray(
            x[b].T.reshape(ET, P, T)[:, :, h * TQ:(h + 1) * TQ]).astype(h16)
        WQKV2, WQKVH = 12288, 6144
        whq = np.ascontiguousarray(
            wcat_h[h][:, b * WQKVH:(b + 1) * WQKVH])
        who = np.ascontiguousarray(
            wcat_h[h][:, WQKV2 + b * 4096:WQKV2 + (b + 1) * 4096])
        bias6 = np.stack([bq[h * KV + ct * P: h * KV + (ct + 1) * P]
                          for ct in range(GROUP)]
                         + [bk[h * D:(h + 1) * D], bv[h * D:(h + 1) * D]],
                         axis=1).astype(f)
        bo4 = np.ascontiguousarray(
            bo[h * KV:(h + 1) * KV].reshape(GROUP, P).T.astype(f))
        per_core.append({
            "xq": xq, "whq": whq, "who": who, "thetas": thetas,
            "bias6": np.ascontiguousarray(bias6), "bo4": bo4,
        })
    return per_core


def kernel(**inputs):
    x = np.asarray(inputs["x"], np.float32)
    nc = _build_program()
    in_maps = _host_inputs(
        x, *(np.asarray(inputs[k], np.float32)
             for k in ("Wq", "bq", "Wk", "bk", "Wv", "bv", "Wo", "bo")))
    res = run_bass_kernel_spmd(nc, in_maps, list(range(N_CORES)))
    out = np.empty((B, T, E), np.float32)
    for b in range(B):
        rows = np.concatenate(
            [res.results[b * GROUP + h]["outt"].reshape(KV, T)
             for h in range(GROUP)], axis=0)                  # [E, T] fp16
        out[b] = rows.T.astype(np.float32)
    return out


# revision 33
# speedup vs baseline: 1.1515x; 1.0424x over previous
"""Grouped-Query Attention (B=2, T=2048, E=2048, 16 Q heads / 4 KV heads, RoPE,
causal) as a Bass/Tile kernel on 8 Trainium2 NeuronCores.

Sharding: core c = 4*b + h handles batch b (of 2) and KV-head group h (of 4,
i.e. 4 q-heads + 1 kv head).  Host->device traffic is minimized (it dominates
the end-to-end time): inputs ship in fp16 with no cross-core duplication and
are reconstructed on device with AllGather collectives:

  - x ships token-quartered (core h gets quarter h of x[b]) -> 4-core AllGather
  - weights ship halved across the batch pair (cores h and h+4 need the same
    head-h weights) -> 2-core AllGather
  - rope tables + causal mask ship 1/8th per core -> 8-core AllGather
  - attention outputs y are AllGathered within each batch group so every core
    computes a distinct 512-row slice of the final out-projection; the output
    is 4x smaller and needs no host-side reduction.

On device everything is channel-major ([channel, token]); matmuls contract
along the partition axis with 512-wide moving operands, fp16 operands with
fp32 PSUM accumulation.
"""

import os
import sys

import numpy as np

try:
    import concourse.bass as bass
except ModuleNotFoundError:  # fresh grading dir: point at the in-container repo
    for p in ("/opt/trn_rl_repo", "/root/.axon_site/_ro/trn_rl_repo"):
        if os.path.isdir(p) and p not in sys.path:
            sys.path.insert(0, p)
    import concourse.bass as bass

from contextlib import ExitStack

import concourse.tile as tile
from concourse import bacc, mybir
from concourse.bass_utils import run_bass_kernel_spmd

# ---- problem constants (hardcoded per contract) ----
B, T, E = 2, 2048, 2048
N_QHEAD, N_KVHEAD = 16, 4
GROUP = N_QHEAD // N_KVHEAD          # 4 q heads per kv head
D = E // N_QHEAD                     # 128 head dim
KV = E // GROUP                      # 512 kv dim
ROPE_BASE = 10000.0
N_CORES = 8

P = 128                              # partitions
ET = E // P                          # 16 e-tiles
TT = T // P                          # 16 token tiles
TC = 512                             # moving-dim chunk (max for fp32 PSUM)
NTC = T // TC                        # 4 token chunks
TQ = T // GROUP                      # 512-token x quarter per core
WCOLS = (ET * GROUP + ET + ET + GROUP * ET) * P   # wq|wk|wv|wo = 20480
WHALF = WCOLS // 2                   # 10240 (= wq|wk and wv|wo exactly)
TME = (3 * T) // N_CORES             # 768 table/mask eighth columns

F32 = mybir.dt.float32
F16 = mybir.dt.float16

G_BATCH = [[0, 1, 2, 3], [4, 5, 6, 7]]           # 4-core batch groups
G_PAIR = [[0, 4], [1, 5], [2, 6], [3, 7]]        # same-head pairs
G_ALL = [list(range(N_CORES))]

_CACHE = {}


def _build_program():
    """Build + compile the (SPMD-identical) Bass program once per process."""
    if "nc" in _CACHE:
        return _CACHE["nc"]

    nc = bacc.Bacc("TRN2", target_bir_lowering=False, debug=False,
                   num_devices=N_CORES)

    dram = {}
    def din(name, shape, dt=F16):
        dram[name] = nc.dram_tensor(name, list(shape), dt,
                                    kind="ExternalInput").ap()
    WQKV = (ET * GROUP + 2 * ET) * P // 2   # 6144: half of [wq|wk|wv]
    WOH = GROUP * ET * P // 2               # 4096: half of wo
    din("xq", (ET, P, TQ))          # token-quarter h of x[b].T, (e, p, t)
    din("whq", (P, WQKV))           # batch-half of [wq|wk|wv] tiles
    din("who", (P, WOH))            # batch-half of wo tiles
    din("thetas", (1, P), F32)      # rope inverse frequencies (row vector)
    din("bias6", (P, 6), F32)       # per-ctile biases: 4x bq, bk, bv
    din("bo4", (P, GROUP), F32)     # bo slice for this core's 4 j-tiles
    outt = nc.dram_tensor("outt", [GROUP, P, T], F16,
                          kind="ExternalOutput").ap()

    with tile.TileContext(nc) as tc:
        with ExitStack() as ctx, nc.allow_low_precision(
                reason="fp16 operands; accumulation stays fp32 in PSUM"):
            dpool = ctx.enter_context(tc.tile_pool(name="dram", bufs=1,
                                                   space="DRAM"))
            persist = ctx.enter_context(tc.tile_pool(name="persist", bufs=1))

            def ptile(shape, name, dt=F16):
                return persist.tile(shape, dt, tag=name, name=name)

            # ---------- DRAM bounce + gathered buffers ----------
            # CC can't read IO tensors -> bounce first.  The weight AllGather
            # goes first (cross-die pairs, D2D-bandwidth-bound); x is gathered
            # in two halves so phase 1 can start after the first half lands;
            # tables gather last (not needed until RoPE).
            NXS = 4                                   # x gathered in 4 slices
            TH = TQ // NXS                            # 128-token slices
            wqb = dpool.tile([P, WQKV], F16)
            wob = dpool.tile([P, WOH], F16)
            xb_s = [dpool.tile([ET, P, TH], F16, name=f"xb{i}")
                    for i in range(NXS)]
            wgq = dpool.tile([2, P, WQKV], F16)
            wgo = dpool.tile([2, P, WOH], F16)
            xg_s = [dpool.tile([GROUP, ET, P, TH], F16, name=f"xg{i}")
                    for i in range(NXS)]              # tokens m*512+[128i,..)
            yb_g = [dpool.tile([P, T], F16, name=f"yb{g}")
                    for g in range(GROUP)]
            yg_g = [dpool.tile([GROUP, P, T], F16, name=f"yg{g}")
                    for g in range(GROUP)]

            nc.sync.dma_start(wqb[:], dram["whq"][:])
            for i in range(NXS):
                nc.sync.dma_start(xb_s[i][:],
                                  dram["xq"][:, :, i * TH:(i + 1) * TH])
            nc.sync.dma_start(wob[:], dram["who"][:])
            # CC order = consumption order: qkv weights, x slices, wo last
            nc.gpsimd.collective_compute(
                "AllGather", mybir.AluOpType.bypass, replica_groups=G_PAIR,
                ins=[wqb.opt()], outs=[wgq.opt()])
            for i in range(NXS):
                nc.gpsimd.collective_compute(
                    "AllGather", mybir.AluOpType.bypass,
                    replica_groups=G_BATCH,
                    ins=[xb_s[i].opt()], outs=[xg_s[i].opt()])
            nc.gpsimd.collective_compute(
                "AllGather", mybir.AluOpType.bypass, replica_groups=G_PAIR,
                ins=[wob.opt()], outs=[wgo.opt()])

            # ---------- persistent SBUF tiles ----------
            wq_sb = ptile([P, ET * GROUP * P], "wq_sb")
            wk_sb = ptile([P, ET * P], "wk_sb")
            wv_sb = ptile([P, ET * P], "wv_sb")
            wo_sb = ptile([P, ET * GROUP * P], "wo_sb")
            bias6_sb = ptile([P, 8], "bias6_sb", F32)  # padded to 32B
            bo4_sb = ptile([P, GROUP], "bo4_sb", F32)
            thetas_sb = ptile([1, P], "thetas_sb", F32)
            ptm_sb = ptile([P, P], "ptm_sb")
            ptm_neg = ptile([P, P], "ptm_neg")
            ident_sb = ptile([P, P], "ident_sb")
            trow_i = ptile([1, T], "trow_i", mybir.dt.int32)
            trow_sb = ptile([1, T], "trow_sb", F32)
            qT_sb = ptile([P, GROUP * T], "qT_sb")    # 4 heads, channel-major
            kT_sb = ptile([P, T], "kT_sb")
            vT_sb = ptile([P, T], "vT_sb")
            vtok_sb = ptile([P, T], "vtok_sb")        # token-major v
            y_sb = ptile([P, GROUP * T], "y_sb")      # yT per head
            cos_sb = ptile([P, T], "cos_sb")
            sin_sb = ptile([P, T], "sin_sb")
            mask4_sb = ptile([P, GROUP * TC], "mask4_sb")
            ones1_sb = ptile([P, 8], "ones1_sb")
            onesr_sb = ptile([1, P], "onesr_sb")

            # pools (SBUF)
            xw = ctx.enter_context(tc.tile_pool(name="xw", bufs=2))    # x / y chunks
            ck = ctx.enter_context(tc.tile_pool(name="ck", bufs=2))    # exp tiles
            osb = ctx.enter_context(tc.tile_pool(name="osb", bufs=2))  # out staging
            # pools (PSUM): statically 4 + 4 = 8 banks
            pacc = ctx.enter_context(tc.tile_pool(name="pacc", bufs=4, space="PSUM"))
            pbig = ctx.enter_context(tc.tile_pool(name="pbig", bufs=2, space="PSUM"))

            # ---------- load + generate constants ----------
            # everything below runs in the shadow of the input AllGathers
            nc.sync.dma_start(bias6_sb[:, 0:6], dram["bias6"][:])
            nc.sync.dma_start(bo4_sb[:], dram["bo4"][:])
            nc.sync.dma_start(thetas_sb[:], dram["thetas"][:])
            nc.vector.memset(ones1_sb[:], 1.0)
            nc.vector.memset(onesr_sb[:], 1.0)

            # identity: 1 where col == p
            nc.gpsimd.memset(ident_sb[:], 1.0)
            nc.gpsimd.affine_select(
                ident_sb[:], ident_sb[:], compare_op=mybir.AluOpType.is_equal,
                fill=0.0, base=0, channel_multiplier=-1, pattern=[[1, P]])
            # rope rotation Pm^T: -1 at col==p-64 (p>=64), +1 at col==p+64
            nc.gpsimd.memset(ptm_sb[:], -1.0)
            nc.gpsimd.affine_select(
                ptm_sb[:], ptm_sb[:], compare_op=mybir.AluOpType.is_equal,
                fill=0.0, base=P // 2, channel_multiplier=-1, pattern=[[1, P]])
            nc.gpsimd.memset(ptm_neg[:], 1.0)
            nc.gpsimd.affine_select(
                ptm_neg[:], ptm_neg[:], compare_op=mybir.AluOpType.is_equal,
                fill=0.0, base=-(P // 2), channel_multiplier=-1,
                pattern=[[1, P]])
            nc.vector.tensor_add(ptm_sb[:], ptm_sb[:], ptm_neg[:])
            # causal masks (4 diagonal-straddle tiles): 1 where t' >= p+128r
            nc.gpsimd.memset(mask4_sb[:], 1.0)
            nc.gpsimd.affine_select(
                mask4_sb[:], mask4_sb[:], compare_op=mybir.AluOpType.is_ge,
                fill=0.0, base=0, channel_multiplier=-1,
                pattern=[[-P, GROUP], [1, TC]])
            # rope tables: ang[p,t] = thetas[p]*(t+1); sin/cos via range
            # reduction to [-pi, pi) and the ACT Sin LUT
            # no mod ALU op on TRN2 -> reduce via k = round(x/2pi) using the
            # round-to-nearest f32->i32 cast, r = x - 2pi*k in [-pi, pi]
            PI = float(np.pi)
            nc.gpsimd.iota(trow_i[:], pattern=[[1, T]], base=1,
                           channel_multiplier=0)
            nc.vector.tensor_copy(trow_sb[:], trow_i[:])
            for c in range(NTC):
                cs = slice(c * TC, (c + 1) * TC)
                aps = pacc.tile([P, TC], F32, tag="acc", name="aps")
                nc.tensor.matmul(aps[:], thetas_sb[:], trow_sb[:, cs],
                                 start=True, stop=True)
                for dst, shift in ((sin_sb, 0.0), (cos_sb, 0.5 * PI)):
                    sc = osb.tile([P, TC], F32, tag="ost", name="sc", bufs=4)
                    yi = osb.tile([P, TC], mybir.dt.int32, tag="yi",
                                  name="yi", bufs=1)
                    yf = osb.tile([P, TC], F32, tag="ost", name="yf", bufs=4)
                    nc.vector.tensor_scalar(sc[:], aps[:], shift,
                                            1.0 / (2 * PI),
                                            mybir.AluOpType.add,
                                            mybir.AluOpType.mult)
                    nc.vector.tensor_copy(yi[:], sc[:])
                    nc.vector.tensor_copy(yf[:], yi[:])
                    nc.vector.tensor_scalar(yf[:], yf[:], -2 * PI, shift,
                                            mybir.AluOpType.mult,
                                            mybir.AluOpType.add)
                    nc.vector.tensor_add(yf[:], aps[:], yf[:])
                    nc.scalar.activation(dst[:, cs], yf[:],
                                         mybir.ActivationFunctionType.Sin)
            # weights from the pair-gathered halves:
            # wgq member 0 = wq tiles [0,48); member 1 = wq[48,64) | wk | wv
            WQC = ET * GROUP * P                      # 8192
            for q3 in range(3):                       # split for DMA parallelism
                s = slice(q3 * WQKV // 3, (q3 + 1) * WQKV // 3)
                nc.sync.dma_start(wq_sb[:, s], wgq[0][:, s])
            nc.sync.dma_start(wq_sb[:, WQKV:WQC], wgq[1][:, 0:WQC - WQKV])
            nc.sync.dma_start(wk_sb[:], wgq[1][:, WQC - WQKV:
                                               WQC - WQKV + ET * P])
            nc.sync.dma_start(wv_sb[:], wgq[1][:, WQC - WQKV + ET * P:WQKV])
            for q4 in range(4):
                s = slice(q4 * WOH // 2, (q4 + 1) * WOH // 2)
                m, off = divmod(q4, 2)
                nc.sync.dma_start(
                    wo_sb[:, s], wgo[m][:, off * WOH // 2:(off + 1) * WOH // 2])
            # ---------- phase 1: QKV projections (channel-major) ----------
            # qT[c,t] = sum_e WqT[e,c] * xT[e,t]  (+bias at evacuation)
            # chunks ordered by gather slice so compute starts on slice 0
            XC = TH                       # 128-token chunks (= x CC slices)

            def proj_dst(ct):
                if ct < GROUP:
                    return qT_sb[:, ct * T:(ct + 1) * T]
                return (kT_sb if ct == GROUP else vT_sb)[:, :]

            for i in range(NXS):
                for mm in range(GROUP):
                    off = mm * TQ + i * TH          # global token offset
                    x_sb = xw.tile([P, ET * XC], F16, tag="xw", name="x_sb")
                    x3 = x_sb[:].rearrange("p (e t) -> p e t", e=ET)
                    xd = xg_s[i][mm][:, :, :].rearrange("e p t -> p e t")
                    for q4 in range(4):
                        nc.sync.dma_start(x3[:, q4 * 4:(q4 + 1) * 4, :],
                                          xd[:, q4 * 4:(q4 + 1) * 4, :])
                    for half in range(2):      # <=3 live PSUM accums at a time
                        for ct3 in range(3):
                            ct = half * 3 + ct3
                            ppr = pacc.tile([P, XC], F32, tag="acc",
                                            name="ppr")
                            for e in range(ET):
                                if ct < GROUP:
                                    lhs = wq_sb[:, (e * GROUP + ct) * P:
                                                (e * GROUP + ct + 1) * P]
                                elif ct == GROUP:
                                    lhs = wk_sb[:, e * P:(e + 1) * P]
                                else:
                                    lhs = wv_sb[:, e * P:(e + 1) * P]
                                nc.tensor.matmul(
                                    ppr[:], lhs,
                                    x_sb[:, e * XC:(e + 1) * XC],
                                    start=(e == 0), stop=(e == ET - 1))
                            dst = proj_dst(ct)
                            nc.vector.tensor_scalar_add(
                                dst[:, off:off + XC], ppr[:],
                                bias6_sb[:, ct:ct + 1])

            # ---------- phase 1b: RoPE (shared tables; k scaled after) ------
            def rope(dst_full):
                for c in range(NTC):
                    cs = slice(c * TC, (c + 1) * TC)
                    rot_ps = pacc.tile([P, TC], F32, tag="acc", name="rot_ps")
                    nc.tensor.matmul(rot_ps[:], ptm_sb[:], dst_full[:, cs],
                                     start=True, stop=True)
                    tmp = osb.tile([P, TC], F32, tag="ost", name="tmp", bufs=4)
                    nc.vector.tensor_mul(tmp[:], rot_ps[:], sin_sb[:, cs])
                    nc.vector.tensor_mul(dst_full[:, cs],
                                         dst_full[:, cs], cos_sb[:, cs])
                    nc.vector.tensor_add(dst_full[:, cs],
                                         dst_full[:, cs], tmp[:])

            rope(kT_sb[:, :])
            # fold the 1/sqrt(D) score scale into k
            nc.vector.tensor_scalar_mul(kT_sb[:], kT_sb[:],
                                        float(1.0 / np.sqrt(D)))

            # ---------- phase 1c: v -> token-major via PE transpose ----------
            for j in range(TT):
                vps = pacc.tile([P, P], F16, tag="acc", name="vps")
                nc.tensor.transpose(vps[:], vT_sb[:, j * P:(j + 1) * P],
                                    ident_sb[:])
                nc.vector.tensor_copy(vtok_sb[:, j * P:(j + 1) * P], vps[:])

            # ---------- phase 2: causal attention per (head, tq-chunk) -------
            # transposed scores: sT[tk, tq] = kT_j^T . qT ; softmax over tk via
            # ones-matmul column sums; normalization folded in at the end.
            # rope of head h+1 (DVE-heavy) overlaps attention of head h
            # (PE-heavy) -- emitted just-in-time per head.
            for h in range(GROUP):
                rope(qT_sb[:, h * T:(h + 1) * T])
                for qc in range(NTC):
                    jmax = GROUP * qc + GROUP - 1
                    ng2 = 2 * (qc + 1)          # groups of 2 j-tiles
                    yps = pacc.tile([P, TC], F32, tag="acc", name="yps")
                    sps = pacc.tile([1, TC], F32, tag="acc", name="sps")

                    def scores(g):
                        # one [128,1024] PSUM tile holding 2 j-tiles' scores
                        spsum = pbig.tile([P, 2 * TC], F32, tag="big",
                                          name="spsum")
                        for sub in range(2):
                            j = 2 * g + sub
                            nc.tensor.matmul(
                                spsum[:, sub * TC:(sub + 1) * TC],
                                kT_sb[:, j * P:(j + 1) * P],
                                qT_sb[:, h * T + qc * TC:
                                      h * T + (qc + 1) * TC],
                                start=True, stop=True)
                        return spsum

                    # software-pipelined: scores of group g+1 are emitted
                    # before the exp/AV consumers of group g so the PE never
                    # sits behind the ACT exp in program order.  The softmax
                    # denominator is accumulated on the DVE (partial column
                    # sums over j-tiles) so only one ones-matmul per q-chunk
                    # runs on the PE instead of one per j-tile.
                    eacc = ck.tile([P, TC], F16, tag="eacc", name="eacc")
                    spsum = scores(0)
                    for g in range(ng2):
                        nxt = scores(g + 1) if g + 1 < ng2 else None
                        eg = ck.tile([P, 2 * TC], F16, tag="ck", name="eg",
                                     bufs=3)
                        nc.scalar.activation(eg[:], spsum[:],
                                             mybir.ActivationFunctionType.Exp)
                        if g >= ng2 - 2:        # diagonal-straddling groups
                            half = g - (ng2 - 2)
                            nc.vector.tensor_mul(
                                eg[:], eg[:],
                                mask4_sb[:, half * 2 * TC:(half + 1) * 2 * TC])
                        # pair-sum on DVE (fast), serial accumulate on the
                        # otherwise-idle GpSimd so neither engine saturates
                        epair = ck.tile([P, TC], F16, tag="epair",
                                        name="epair", bufs=3)
                        nc.vector.tensor_add(epair[:], eg[:, 0:TC],
                                             eg[:, TC:2 * TC])
                        if g == 0:
                            nc.gpsimd.tensor_copy(eacc[:], epair[:])
                        else:
                            nc.gpsimd.tensor_add(eacc[:], eacc[:], epair[:])
                        for sub in range(2):
                            j = 2 * g + sub
                            nc.tensor.matmul(
                                yps[:], vtok_sb[:, j * P:(j + 1) * P],
                                eg[:, sub * TC:(sub + 1) * TC],
                                start=(j == 0), stop=(j == jmax))
                        spsum = nxt
                    nc.tensor.matmul(sps[:], ones1_sb[:, 0:1], eacc[:],
                                     start=True, stop=True)
                    # normalize: y /= colsum (broadcast 1/sum via K=1 matmul)
                    rec = osb.tile([1, TC], F16, tag="rec", name="rec", bufs=1)
                    nc.vector.reciprocal(rec[:], sps[:])
                    bps = pacc.tile([P, TC], F32, tag="acc", name="bps")
                    nc.tensor.matmul(bps[:], onesr_sb[:], rec[:],
                                     start=True, stop=True)
                    bcs = osb.tile([P, TC], F32, tag="bc", name="bcs", bufs=1)
                    nc.scalar.copy(bcs[:], bps[:])
                    nc.vector.tensor_mul(
                        y_sb[:, h * T + qc * TC: h * T + (qc + 1) * TC],
                        yps[:], bcs[:])
                # stage + gather this head's y now: the CC overlaps the next
                # head's attention; only head 3's gather sits on the tail
                nc.sync.dma_start(yb_g[h][:], y_sb[:, h * T:(h + 1) * T])
                nc.gpsimd.collective_compute(
                    "AllGather", mybir.AluOpType.bypass,
                    replica_groups=G_BATCH,
                    ins=[yb_g[h].opt()], outs=[yg_g[h].opt()])

            # ---------- phase 3: out-projection rows [h*512,(h+1)*512) ------
            # outT[j,t] = sum_c WoS[j,c] * yT[c,t] + bo[j]; y gathered from
            # all 4 cores of this batch group, streamed by token chunk.
            for c in range(NTC):
                ysb = xw.tile([P, ET * TC], F16, tag="ysb", name="ysb",
                              bufs=3)
                y4 = ysb[:].rearrange("p (h g t) -> p h g t", h=GROUP, g=GROUP)
                for g in range(GROUP):
                    src = yg_g[g][:, :, c * TC:(c + 1) * TC].rearrange(
                        "h d t -> d h t")
                    nc.sync.dma_start(y4[:, :, g, :], src)
                for jj in range(GROUP):
                    ops = pacc.tile([P, TC], F32, tag="acc", name="ops")
                    for ct in range(ET):
                        nc.tensor.matmul(
                            ops[:], wo_sb[:, (ct * GROUP + jj) * P:
                                          (ct * GROUP + jj + 1) * P],
                            ysb[:, ct * TC:(ct + 1) * TC],
                            start=(ct == 0), stop=(ct == ET - 1))
                    ost = osb.tile([P, TC], F16, tag="ost2", name="ost", bufs=4)
                    nc.vector.tensor_scalar_add(ost[:], ops[:],
                                                bo4_sb[:, jj:jj + 1])
                    nc.sync.dma_start(outt[jj][:, c * TC:(c + 1) * TC], ost[:])

    nc.compile()
    _CACHE["nc"] = nc
    return nc


def _host_inputs(x, Wq, bq, Wk, bk, Wv, bv, Wo, bo):
    """Per-core input dicts (fp16 payloads, layouts matching the DRAM decls)."""
    f = np.float32
    h16 = np.float16
    i = np.arange(D // 2, dtype=np.float64)
    th_half = ROPE_BASE ** (-2.0 * i / D)
    thetas = np.concatenate([th_half, th_half]).astype(f).reshape(1, P)

    # per-head weight blocks [wq|wk|wv|wo] -> [P, 20480] fp16
    wcat_h = []
    for h in range(GROUP):
        WqS = Wq[h * KV:(h + 1) * KV, :]                                # [512, E]
        wq = WqS.T.reshape(ET, P, GROUP, P).transpose(1, 0, 2, 3).reshape(P, -1)
        WkS = Wk[h * D:(h + 1) * D, :]
        wk = WkS.T.reshape(ET, P, P).transpose(1, 0, 2).reshape(P, -1)
        WvS = Wv[h * D:(h + 1) * D, :]
        wv = WvS.T.reshape(ET, P, P).transpose(1, 0, 2).reshape(P, -1)
        WoS = Wo[h * KV:(h + 1) * KV, :]                                # [512, E]
        wo = WoS.reshape(GROUP, P, ET, P).transpose(3, 2, 0, 1).reshape(P, -1)
        wcat_h.append(np.concatenate([wq, wk, wv, wo], axis=1).astype(h16))

    per_core = []
    for c in range(N_CORES):
        b, h = divmod(c, GROUP)
        xq = np.ascontiguousarray(
            x[b].T.reshape(ET, P, T)[:, :, h * TQ:(h + 1) * TQ]).astype(h16)
        WQKV2, WQKVH = 12288, 6144
        whq = np.ascontiguousarray(
            wcat_h[h][:, b * WQKVH:(b + 1) * WQKVH])
        who = np.ascontiguousarray(
            wcat_h[h][:, WQKV2 + b * 4096:WQKV2 + (b + 1) * 4096])
        bias6 = np.stack([bq[h * KV + ct * P: h * KV + (ct + 1) * P]
                          for ct in range(GROUP)]
                         + [bk[h * D:(h + 1) * D], bv[h * D:(h + 1) * D]],
                         axis=1).astype(f)
        bo4 = np.ascontiguousarray(
            bo[h * KV:(h + 1) * KV].reshape(GROUP, P).T.astype(f))
        per_core.append({
            "xq": xq, "whq": whq, "who": who, "thetas": thetas,
            "bias6": np.ascontiguousarray(bias6), "bo4": bo4,
        })
    return per_core


def kernel(**inputs):
    x = np.asarray(inputs["x"], np.float32)
    nc = _build_program()
    in_maps = _host_inputs(
        x, *(np.asarray(inputs[k], np.float32)
             for k in ("Wq", "bq", "Wk", "bk", "Wv", "bv", "Wo", "bo")))
    res = run_bass_kernel_spmd(nc, in_maps, list(range(N_CORES)))
    out = np.empty((B, T, E), np.float32)
    for b in range(B):
        rows = np.concatenate(
            [res.results[b * GROUP + h]["outt"].reshape(KV, T)
             for h in range(GROUP)], axis=0)                  # [E, T] fp16
        out[b] = rows.T.astype(np.float32)
    return out


# revision 34
# speedup vs baseline: 1.1754x; 1.0207x over previous
"""Grouped-Query Attention (B=2, T=2048, E=2048, 16 Q heads / 4 KV heads, RoPE,
causal) as a Bass/Tile kernel on 8 Trainium2 NeuronCores.

Sharding: core c = 4*b + h handles batch b (of 2) and KV-head group h (of 4,
i.e. 4 q-heads + 1 kv head).  Host->device traffic is minimized (it dominates
the end-to-end time): inputs ship in fp16 with no cross-core duplication and
are reconstructed on device with AllGather collectives:

  - x ships token-quartered (core h gets quarter h of x[b]) -> 4-core AllGather
  - weights ship halved across the batch pair (cores h and h+4 need the same
    head-h weights) -> 2-core AllGather
  - rope tables + causal mask ship 1/8th per core -> 8-core AllGather
  - attention outputs y are AllGathered within each batch group so every core
    computes a distinct 512-row slice of the final out-projection; the output
    is 4x smaller and needs no host-side reduction.

On device everything is channel-major ([channel, token]); matmuls contract
along the partition axis with 512-wide moving operands, fp16 operands with
fp32 PSUM accumulation.
"""

import os
import sys

import numpy as np

try:
    import concourse.bass as bass
except ModuleNotFoundError:  # fresh grading dir: point at the in-container repo
    for p in ("/opt/trn_rl_repo", "/root/.axon_site/_ro/trn_rl_repo"):
        if os.path.isdir(p) and p not in sys.path:
            sys.path.insert(0, p)
    import concourse.bass as bass

from contextlib import ExitStack

import concourse.tile as tile
from concourse import bacc, mybir
from concourse.bass_utils import run_bass_kernel_spmd

# ---- problem constants (hardcoded per contract) ----
B, T, E = 2, 2048, 2048
N_QHEAD, N_KVHEAD = 16, 4
GROUP = N_QHEAD // N_KVHEAD          # 4 q heads per kv head
D = E // N_QHEAD                     # 128 head dim
KV = E // GROUP                      # 512 kv dim
ROPE_BASE = 10000.0
N_CORES = 8

P = 128                              # partitions
ET = E // P                          # 16 e-tiles
TT = T // P                          # 16 token tiles
TC = 512                             # moving-dim chunk (max for fp32 PSUM)
NTC = T // TC                        # 4 token chunks
TQ = T // GROUP                      # 512-token x quarter per core
WCOLS = (ET * GROUP + ET + ET + GROUP * ET) * P   # wq|wk|wv|wo = 20480
WHALF = WCOLS // 2                   # 10240 (= wq|wk and wv|wo exactly)
TME = (3 * T) // N_CORES             # 768 table/mask eighth columns

F32 = mybir.dt.float32
F16 = mybir.dt.float16

G_BATCH = [[0, 1, 2, 3], [4, 5, 6, 7]]           # 4-core batch groups
G_PAIR = [[0, 4], [1, 5], [2, 6], [3, 7]]        # same-head pairs
G_ALL = [list(range(N_CORES))]

_CACHE = {}


def _build_program():
    """Build + compile the (SPMD-identical) Bass program once per process."""
    if "nc" in _CACHE:
        return _CACHE["nc"]

    nc = bacc.Bacc("TRN2", target_bir_lowering=False, debug=False,
                   num_devices=N_CORES)

    dram = {}
    def din(name, shape, dt=F16):
        dram[name] = nc.dram_tensor(name, list(shape), dt,
                                    kind="ExternalInput").ap()
    WQKV = (ET * GROUP + 2 * ET) * P // 2   # 6144: half of [wq|wk|wv]
    WOH = GROUP * ET * P // 2               # 4096: half of wo
    din("xq", (ET, P, TQ))          # token-quarter h of x[b].T, (e, p, t)
    din("whq", (P, WQKV))           # batch-half of [wq|wk|wv] tiles
    din("who", (P, WOH))            # batch-half of wo tiles
    din("thetas", (1, P), F32)      # rope inverse frequencies (row vector)
    din("bias6", (P, 6), F32)       # per-ctile biases: 4x bq, bk, bv
    din("bo4", (P, GROUP), F32)     # bo slice for this core's 4 j-tiles
    outt = nc.dram_tensor("outt", [GROUP, P, T], F16,
                          kind="ExternalOutput").ap()

    with tile.TileContext(nc) as tc:
        with ExitStack() as ctx, nc.allow_low_precision(
                reason="fp16 operands; accumulation stays fp32 in PSUM"):
            dpool = ctx.enter_context(tc.tile_pool(name="dram", bufs=1,
                                                   space="DRAM"))
            persist = ctx.enter_context(tc.tile_pool(name="persist", bufs=1))

            def ptile(shape, name, dt=F16):
                return persist.tile(shape, dt, tag=name, name=name)

            # ---------- DRAM bounce + gathered buffers ----------
            # CC can't read IO tensors -> bounce first.  The weight AllGather
            # goes first (cross-die pairs, D2D-bandwidth-bound); x is gathered
            # in two halves so phase 1 can start after the first half lands;
            # tables gather last (not needed until RoPE).
            NXS = 4                                   # x gathered in 4 slices
            TH = TQ // NXS                            # 128-token slices
            wqb = dpool.tile([P, WQKV], F16)
            wob = dpool.tile([P, WOH], F16)
            xb_s = [dpool.tile([ET, P, TH], F16, name=f"xb{i}")
                    for i in range(NXS)]
            wgq = dpool.tile([2, P, WQKV], F16)
            wgo = dpool.tile([2, P, WOH], F16)
            xg_s = [dpool.tile([GROUP, ET, P, TH], F16, name=f"xg{i}")
                    for i in range(NXS)]              # tokens m*512+[128i,..)
            yb_g = [dpool.tile([P, T], F16, name=f"yb{g}")
                    for g in range(GROUP)]
            yg_g = [dpool.tile([GROUP, P, T], F16, name=f"yg{g}")
                    for g in range(GROUP)]

            nc.sync.dma_start(wqb[:], dram["whq"][:])
            for i in range(NXS):
                nc.sync.dma_start(xb_s[i][:],
                                  dram["xq"][:, :, i * TH:(i + 1) * TH])
            nc.sync.dma_start(wob[:], dram["who"][:])
            # CC order = consumption order: qkv weights, x slices, wo last
            nc.gpsimd.collective_compute(
                "AllGather", mybir.AluOpType.bypass, replica_groups=G_PAIR,
                ins=[wqb.opt()], outs=[wgq.opt()])
            for i in range(NXS):
                nc.gpsimd.collective_compute(
                    "AllGather", mybir.AluOpType.bypass,
                    replica_groups=G_BATCH,
                    ins=[xb_s[i].opt()], outs=[xg_s[i].opt()])
            nc.gpsimd.collective_compute(
                "AllGather", mybir.AluOpType.bypass, replica_groups=G_PAIR,
                ins=[wob.opt()], outs=[wgo.opt()])

            # ---------- persistent SBUF tiles ----------
            wq_sb = ptile([P, ET * GROUP * P], "wq_sb")
            wk_sb = ptile([P, ET * P], "wk_sb")
            wv_sb = ptile([P, ET * P], "wv_sb")
            wo_sb = ptile([P, ET * GROUP * P], "wo_sb")
            bias6_sb = ptile([P, 8], "bias6_sb", F32)  # padded to 32B
            bo4_sb = ptile([P, GROUP], "bo4_sb", F32)
            thetas_sb = ptile([1, P], "thetas_sb", F32)
            ptm_sb = ptile([P, P], "ptm_sb")
            ptm_neg = ptile([P, P], "ptm_neg")
            ident_sb = ptile([P, P], "ident_sb")
            trow_i = ptile([1, T], "trow_i", mybir.dt.int32)
            trow_sb = ptile([1, T], "trow_sb", F32)
            qT_sb = ptile([P, GROUP * T], "qT_sb")    # 4 heads, channel-major
            kT_sb = ptile([P, T], "kT_sb")
            vT_sb = ptile([P, T], "vT_sb")
            vtok_sb = ptile([P, T], "vtok_sb")        # token-major v
            y_sb = ptile([P, GROUP * T], "y_sb")      # yT per head
            cos_sb = ptile([P, T], "cos_sb")
            sin_sb = ptile([P, T], "sin_sb")
            mask4_sb = ptile([P, GROUP * TC], "mask4_sb")
            ones1_sb = ptile([P, 8], "ones1_sb")
            onesr_sb = ptile([1, P], "onesr_sb")

            # pools (SBUF)
            xw = ctx.enter_context(tc.tile_pool(name="xw", bufs=2))    # x / y chunks
            ck = ctx.enter_context(tc.tile_pool(name="ck", bufs=2))    # exp tiles
            osb = ctx.enter_context(tc.tile_pool(name="osb", bufs=2))  # out staging
            # pools (PSUM): statically 4 + 4 = 8 banks
            pacc = ctx.enter_context(tc.tile_pool(name="pacc", bufs=4, space="PSUM"))
            pbig = ctx.enter_context(tc.tile_pool(name="pbig", bufs=2, space="PSUM"))

            # ---------- load + generate constants ----------
            # everything below runs in the shadow of the input AllGathers
            nc.sync.dma_start(bias6_sb[:, 0:6], dram["bias6"][:])
            nc.sync.dma_start(bo4_sb[:], dram["bo4"][:])
            nc.sync.dma_start(thetas_sb[:], dram["thetas"][:])
            nc.vector.memset(ones1_sb[:], 1.0)
            nc.vector.memset(onesr_sb[:], 1.0)

            # identity: 1 where col == p
            nc.gpsimd.memset(ident_sb[:], 1.0)
            nc.gpsimd.affine_select(
                ident_sb[:], ident_sb[:], compare_op=mybir.AluOpType.is_equal,
                fill=0.0, base=0, channel_multiplier=-1, pattern=[[1, P]])
            # rope rotation Pm^T: -1 at col==p-64 (p>=64), +1 at col==p+64
            nc.gpsimd.memset(ptm_sb[:], -1.0)
            nc.gpsimd.affine_select(
                ptm_sb[:], ptm_sb[:], compare_op=mybir.AluOpType.is_equal,
                fill=0.0, base=P // 2, channel_multiplier=-1, pattern=[[1, P]])
            nc.gpsimd.memset(ptm_neg[:], 1.0)
            nc.gpsimd.affine_select(
                ptm_neg[:], ptm_neg[:], compare_op=mybir.AluOpType.is_equal,
                fill=0.0, base=-(P // 2), channel_multiplier=-1,
                pattern=[[1, P]])
            nc.vector.tensor_add(ptm_sb[:], ptm_sb[:], ptm_neg[:])
            # causal masks (4 diagonal-straddle tiles): 1 where t' >= p+128r
            nc.gpsimd.memset(mask4_sb[:], 1.0)
            nc.gpsimd.affine_select(
                mask4_sb[:], mask4_sb[:], compare_op=mybir.AluOpType.is_ge,
                fill=0.0, base=0, channel_multiplier=-1,
                pattern=[[-P, GROUP], [1, TC]])
            # rope tables: ang[p,t] = thetas[p]*(t+1); sin/cos via range
            # reduction to [-pi, pi) and the ACT Sin LUT
            # no mod ALU op on TRN2 -> reduce via k = round(x/2pi) using the
            # round-to-nearest f32->i32 cast, r = x - 2pi*k in [-pi, pi]
            PI = float(np.pi)
            nc.gpsimd.iota(trow_i[:], pattern=[[1, T]], base=1,
                           channel_multiplier=0)
            nc.vector.tensor_copy(trow_sb[:], trow_i[:])
            for c in range(NTC):
                cs = slice(c * TC, (c + 1) * TC)
                aps = pacc.tile([P, TC], F32, tag="acc", name="aps")
                nc.tensor.matmul(aps[:], thetas_sb[:], trow_sb[:, cs],
                                 start=True, stop=True)
                for dst, shift in ((sin_sb, 0.0), (cos_sb, 0.5 * PI)):
                    sc = osb.tile([P, TC], F32, tag="ost", name="sc", bufs=4)
                    yi = osb.tile([P, TC], mybir.dt.int32, tag="yi",
                                  name="yi", bufs=1)
                    yf = osb.tile([P, TC], F32, tag="ost", name="yf", bufs=4)
                    nc.vector.tensor_scalar(sc[:], aps[:], shift,
                                            1.0 / (2 * PI),
                                            mybir.AluOpType.add,
                                            mybir.AluOpType.mult)
                    nc.vector.tensor_copy(yi[:], sc[:])
                    nc.vector.tensor_copy(yf[:], yi[:])
                    nc.vector.tensor_scalar(yf[:], yf[:], -2 * PI, shift,
                                            mybir.AluOpType.mult,
                                            mybir.AluOpType.add)
                    nc.vector.tensor_add(yf[:], aps[:], yf[:])
                    nc.scalar.activation(dst[:, cs], yf[:],
                                         mybir.ActivationFunctionType.Sin)
            # weights from the pair-gathered halves:
            # wgq member 0 = wq tiles [0,48); member 1 = wq[48,64) | wk | wv
            WQC = ET * GROUP * P                      # 8192
            for q3 in range(3):                       # split for DMA parallelism
                s = slice(q3 * WQKV // 3, (q3 + 1) * WQKV // 3)
                nc.sync.dma_start(wq_sb[:, s], wgq[0][:, s])
            nc.sync.dma_start(wq_sb[:, WQKV:WQC], wgq[1][:, 0:WQC - WQKV])
            nc.sync.dma_start(wk_sb[:], wgq[1][:, WQC - WQKV:
                                               WQC - WQKV + ET * P])
            nc.sync.dma_start(wv_sb[:], wgq[1][:, WQC - WQKV + ET * P:WQKV])
            for q4 in range(4):
                s = slice(q4 * WOH // 2, (q4 + 1) * WOH // 2)
                m, off = divmod(q4, 2)
                nc.sync.dma_start(
                    wo_sb[:, s], wgo[m][:, off * WOH // 2:(off + 1) * WOH // 2])
            # ---------- phase 1: QKV projections (channel-major) ----------
            # qT[c,t] = sum_e WqT[e,c] * xT[e,t]  (+bias at evacuation)
            # chunks ordered by gather slice so compute starts on slice 0
            XC = TH                       # 128-token chunks (= x CC slices)

            def proj_dst(ct):
                if ct < GROUP:
                    return qT_sb[:, ct * T:(ct + 1) * T]
                return (kT_sb if ct == GROUP else vT_sb)[:, :]

            for i in range(NXS):
                for mm in range(GROUP):
                    off = mm * TQ + i * TH          # global token offset
                    x_sb = xw.tile([P, ET * XC], F16, tag="xw", name="x_sb")
                    x3 = x_sb[:].rearrange("p (e t) -> p e t", e=ET)
                    xd = xg_s[i][mm][:, :, :].rearrange("e p t -> p e t")
                    for q4 in range(4):
                        nc.sync.dma_start(x3[:, q4 * 4:(q4 + 1) * 4, :],
                                          xd[:, q4 * 4:(q4 + 1) * 4, :])
                    for half in range(2):      # <=3 live PSUM accums at a time
                        for ct3 in range(3):
                            ct = half * 3 + ct3
                            ppr = pacc.tile([P, XC], F32, tag="acc",
                                            name="ppr")
                            for e in range(ET):
                                if ct < GROUP:
                                    lhs = wq_sb[:, (e * GROUP + ct) * P:
                                                (e * GROUP + ct + 1) * P]
                                elif ct == GROUP:
                                    lhs = wk_sb[:, e * P:(e + 1) * P]
                                else:
                                    lhs = wv_sb[:, e * P:(e + 1) * P]
                                nc.tensor.matmul(
                                    ppr[:], lhs,
                                    x_sb[:, e * XC:(e + 1) * XC],
                                    start=(e == 0), stop=(e == ET - 1))
                            dst = proj_dst(ct)
                            nc.vector.tensor_scalar_add(
                                dst[:, off:off + XC], ppr[:],
                                bias6_sb[:, ct:ct + 1])

            # ---------- phase 1b: RoPE (shared tables; k scaled after) ------
            def rope(dst_full):
                for c in range(NTC):
                    cs = slice(c * TC, (c + 1) * TC)
                    rot_ps = pacc.tile([P, TC], F32, tag="acc", name="rot_ps")
                    nc.tensor.matmul(rot_ps[:], ptm_sb[:], dst_full[:, cs],
                                     start=True, stop=True)
                    tmp = osb.tile([P, TC], F32, tag="ost", name="tmp", bufs=4)
                    nc.vector.tensor_mul(tmp[:], rot_ps[:], sin_sb[:, cs])
                    nc.vector.tensor_mul(dst_full[:, cs],
                                         dst_full[:, cs], cos_sb[:, cs])
                    nc.vector.tensor_add(dst_full[:, cs],
                                         dst_full[:, cs], tmp[:])

            rope(kT_sb[:, :])
            # fold the 1/sqrt(D) score scale into k
            nc.vector.tensor_scalar_mul(kT_sb[:], kT_sb[:],
                                        float(1.0 / np.sqrt(D)))

            # ---------- phase 1c: v -> token-major via PE transpose ----------
            for j in range(TT):
                vps = pacc.tile([P, P], F16, tag="acc", name="vps")
                nc.tensor.transpose(vps[:], vT_sb[:, j * P:(j + 1) * P],
                                    ident_sb[:])
                nc.vector.tensor_copy(vtok_sb[:, j * P:(j + 1) * P], vps[:])

            # ---------- phase 2: causal attention per (head, tq-chunk) -------
            # transposed scores: sT[tk, tq] = kT_j^T . qT ; softmax over tk via
            # ones-matmul column sums; normalization folded in at the end.
            # rope of head h+1 (DVE-heavy) overlaps attention of head h
            # (PE-heavy) -- emitted just-in-time per head.
            for h in range(GROUP):
                rope(qT_sb[:, h * T:(h + 1) * T])
                for qc in range(NTC):
                    jmax = GROUP * qc + GROUP - 1
                    ng2 = 2 * (qc + 1)          # groups of 2 j-tiles
                    yps = pacc.tile([P, TC], F32, tag="acc", name="yps")
                    sps = pacc.tile([1, TC], F32, tag="acc", name="sps")

                    def scores(g):
                        # one [128,1024] PSUM tile holding 2 j-tiles' scores
                        spsum = pbig.tile([P, 2 * TC], F32, tag="big",
                                          name="spsum")
                        for sub in range(2):
                            j = 2 * g + sub
                            nc.tensor.matmul(
                                spsum[:, sub * TC:(sub + 1) * TC],
                                kT_sb[:, j * P:(j + 1) * P],
                                qT_sb[:, h * T + qc * TC:
                                      h * T + (qc + 1) * TC],
                                start=True, stop=True)
                        return spsum

                    # software-pipelined: scores of group g+1 are emitted
                    # before the exp/AV consumers of group g so the PE never
                    # sits behind the ACT exp in program order.  The softmax
                    # denominator is accumulated on the DVE (partial column
                    # sums over j-tiles) so only one ones-matmul per q-chunk
                    # runs on the PE instead of one per j-tile.
                    eacc = ck.tile([P, TC], F16, tag="eacc", name="eacc")
                    spsum = scores(0)
                    for g in range(ng2):
                        nxt = scores(g + 1) if g + 1 < ng2 else None
                        eg = ck.tile([P, 2 * TC], F16, tag="ck", name="eg",
                                     bufs=3)
                        nc.scalar.activation(eg[:], spsum[:],
                                             mybir.ActivationFunctionType.Exp)
                        if g >= ng2 - 2:        # diagonal-straddling groups
                            half = g - (ng2 - 2)
                            nc.vector.tensor_mul(
                                eg[:], eg[:],
                                mask4_sb[:, half * 2 * TC:(half + 1) * 2 * TC])
                        # pair-sum on DVE (fast), serial accumulate on the
                        # otherwise-idle GpSimd so neither engine saturates
                        epair = ck.tile([P, TC], F16, tag="epair",
                                        name="epair", bufs=3)
                        nc.vector.tensor_add(epair[:], eg[:, 0:TC],
                                             eg[:, TC:2 * TC])
                        if g == 0:
                            nc.gpsimd.tensor_copy(eacc[:], epair[:])
                        else:
                            nc.gpsimd.tensor_add(eacc[:], eacc[:], epair[:])
                        for sub in range(2):
                            j = 2 * g + sub
                            nc.tensor.matmul(
                                yps[:], vtok_sb[:, j * P:(j + 1) * P],
                                eg[:, sub * TC:(sub + 1) * TC],
                                start=(j == 0), stop=(j == jmax))
                        spsum = nxt
                    nc.tensor.matmul(sps[:], ones1_sb[:, 0:1], eacc[:],
                                     start=True, stop=True)
                    # normalize: y /= colsum (broadcast 1/sum via K=1 matmul)
                    rec = osb.tile([1, TC], F16, tag="rec", name="rec", bufs=1)
                    nc.vector.reciprocal(rec[:], sps[:])
                    bps = pacc.tile([P, TC], F32, tag="acc", name="bps")
                    nc.tensor.matmul(bps[:], onesr_sb[:], rec[:],
                                     start=True, stop=True)
                    bcs = osb.tile([P, TC], F32, tag="bc", name="bcs", bufs=1)
                    nc.scalar.copy(bcs[:], bps[:])
                    nc.vector.tensor_mul(
                        y_sb[:, h * T + qc * TC: h * T + (qc + 1) * TC],
                        yps[:], bcs[:])
                # stage + gather this head's y now: the CC overlaps the next
                # head's attention; only head 3's gather sits on the tail
                nc.sync.dma_start(yb_g[h][:], y_sb[:, h * T:(h + 1) * T])
                nc.gpsimd.collective_compute(
                    "AllGather", mybir.AluOpType.bypass,
                    replica_groups=G_BATCH,
                    ins=[yb_g[h].opt()], outs=[yg_g[h].opt()])

            # ---------- phase 3: out-projection rows [h*512,(h+1)*512) ------
            # outT[j,t] = sum_c WoS[j,c] * yT[c,t] + bo[j]; y gathered from
            # all 4 cores of this batch group, streamed by token chunk.
            for c in range(NTC):
                ysb = xw.tile([P, ET * TC], F16, tag="ysb", name="ysb",
                              bufs=3)
                y4 = ysb[:].rearrange("p (h g t) -> p h g t", h=GROUP, g=GROUP)
                for g in range(GROUP):
                    src = yg_g[g][:, :, c * TC:(c + 1) * TC].rearrange(
                        "h d t -> d h t")
                    nc.sync.dma_start(y4[:, :, g, :], src)
                # contract head-3 tiles (ct%4==3) last: the first 12 matmuls
                # per output tile only need y heads 0-2, so the PE overlaps
                # head 3's y AllGather instead of idling behind it
                ct_early = [ct for ct in range(ET) if ct % GROUP != 3]
                ct_late = [ct for ct in range(ET) if ct % GROUP == 3]
                ops_jj = []
                for jj in range(GROUP):
                    ops = pacc.tile([P, TC], F32, tag="acc", name="ops")
                    ops_jj.append(ops)
                    for k, ct in enumerate(ct_early):
                        nc.tensor.matmul(
                            ops[:], wo_sb[:, (ct * GROUP + jj) * P:
                                          (ct * GROUP + jj + 1) * P],
                            ysb[:, ct * TC:(ct + 1) * TC],
                            start=(k == 0), stop=False)
                for jj in range(GROUP):
                    ops = ops_jj[jj]
                    for k, ct in enumerate(ct_late):
                        nc.tensor.matmul(
                            ops[:], wo_sb[:, (ct * GROUP + jj) * P:
                                          (ct * GROUP + jj + 1) * P],
                            ysb[:, ct * TC:(ct + 1) * TC],
                            start=False, stop=(k == len(ct_late) - 1))
                    ost = osb.tile([P, TC], F16, tag="ost2", name="ost", bufs=4)
                    nc.vector.tensor_scalar_add(ost[:], ops[:],
                                                bo4_sb[:, jj:jj + 1])
                    nc.sync.dma_start(outt[jj][:, c * TC:(c + 1) * TC], ost[:])

    nc.compile()
    _CACHE["nc"] = nc
    return nc


def _host_inputs(x, Wq, bq, Wk, bk, Wv, bv, Wo, bo):
    """Per-core input dicts (fp16 payloads, layouts matching the DRAM decls)."""
    f = np.float32
    h16 = np.float16
    i = np.arange(D // 2, dtype=np.float64)
    th_half = ROPE_BASE ** (-2.0 * i / D)
    thetas = np.concatenate([th_half, th_half]).astype(f).reshape(1, P)

    # per-head weight blocks [wq|wk|wv|wo] -> [P, 20480] fp16
    wcat_h = []
    for h in range(GROUP):
        WqS = Wq[h * KV:(h + 1) * KV, :]                                # [512, E]
        wq = WqS.T.reshape(ET, P, GROUP, P).transpose(1, 0, 2, 3).reshape(P, -1)
        WkS = Wk[h * D:(h + 1) * D, :]
        wk = WkS.T.reshape(ET, P, P).transpose(1, 0, 2).reshape(P, -1)
        WvS = Wv[h * D:(h + 1) * D, :]
        wv = WvS.T.reshape(ET, P, P).transpose(1, 0, 2).reshape(P, -1)
        WoS = Wo[h * KV:(h + 1) * KV, :]                                # [512, E]
        wo = WoS.reshape(GROUP, P, ET, P).transpose(3, 2, 0, 1).reshape(P, -1)
        wcat_h.append(np.concatenate([wq, wk, wv, wo], axis=1).astype(h16))

    per_core = []
    for c in range(N_CORES):
        b, h = divmod(c, GROUP)
        xq = np.ascontiguousarray(
            x[b].T.reshape(ET, P, T)[:, :, h * TQ:(h + 1) * TQ]).astype(h16)
        WQKV2, WQKVH = 12288, 6144
        whq = np.ascontiguousarray(
            wcat_h[h][:, b * WQKVH:(b + 1) * WQKVH])
        who = np.ascontiguousarray(
            wcat_h[h][:, WQKV2 + b * 4096:WQKV2 + (b + 1) * 4096])
        bias6 = np.stack([bq[h * KV + ct * P: h * KV + (ct + 1) * P]
                          for ct in range(GROUP)]
                         + [bk[h * D:(h + 1) * D], bv[h * D:(h + 1) * D]],
                         axis=1).astype(f)
        bo4 = np.ascontiguousarray(
            bo[h * KV:(h + 1) * KV].reshape(GROUP, P).T.astype(f))
        per_core.append({
            "xq": xq, "whq": whq, "who": who, "thetas": thetas,
            "bias6": np.ascontiguousarray(bias6), "bo4": bo4,
        })
    return per_core


def kernel(**inputs):
    x = np.asarray(inputs["x"], np.float32)
    nc = _build_program()
    in_maps = _host_inputs(
        x, *(np.asarray(inputs[k], np.float32)
             for k in ("Wq", "bq", "Wk", "bk", "Wv", "bv", "Wo", "bo")))
    res = run_bass_kernel_spmd(nc, in_maps, list(range(N_CORES)))
    out = np.empty((B, T, E), np.float32)
    for b in range(B):
        rows = np.concatenate(
            [res.results[b * GROUP + h]["outt"].reshape(KV, T)
             for h in range(GROUP)], axis=0)                  # [E, T] fp16
        out[b] = rows.T.astype(np.float32)
    return out
